# revision 21
# baseline (speedup 1.0000x reference)
"""Causal self-attention (B=4, T=2048, C=1024, H=16) on 8 trn2 NeuronCores.

Sharding: core = (batch b, head-group g), b in 0..3, g in 0..1. Each core does
8 heads of one batch element (Megatron column split of w_attn, row split of
w_proj); host sums the two partial projection outputs per batch element.

Per-core kernel, v4 (reduced PE work + coarse DMA):
 - All DRAM inputs bf16 (host casts); attention matmul operands bf16.
 - Coarse multi-c-block DMA descriptors issued in parallel from four engine
   queues at start (the v3 per-chunk DMAs serialized ~600ns/issue on sync and
   starved the V phase).
 - Q^T,K^T computed transposed (lhsT=W-block, rhs=x^T-block) so attention
   needs no transposes; V natural with a ones column per head so the
   attention AV matmul accumulates the softmax denominator l for free.
 - Attention per head-pair: S^T for both heads row-tiled into one
   [128,1024] PSUM tile per k-block; one exp (scale=1/8 folded in, no
   max-subtraction -- scores are N(0,1)); causal mask only on diagonal
   blocks via one doubled-mask bf16 multiply ON GPSIMD; AV deferred four
   k-blocks so exp latency hides; filler units (next head-pair's QK
   projection, output projection) credit-paced into every k-block.
 - qc processed descending (3,2,1,0): the long qc pipelines come first and
   the tiny qc0 flush lands where fillers still exist; hp3's projection
   tail units enqueue one qc earlier.
 - Normalization: l rows copied off PSUM + reciprocal on DVE, then ONE
   fp32r matmul per qc (contraction-2 selection matrix) builds the [128,512]
   broadcast of 1/l for BOTH heads (v3 used two rank-1 matmuls), and the Y
   drain is fused with the 1/l multiply in a single DVE tensor_tensor op
   per head (replaces v3's separate copy + in-place multiply).  The norm
   unit is pushed to the FRONT of the filler queue so the Y PSUM frees
   within a k-block or two.  (partition_broadcast was tried and reverted:
   it lives in a different gpsimd ucode library than tensor_tensor, and
   the resulting per-qc library swaps cost ~14us each.)
"""

import sys

if "/opt/trn_rl_repo" not in sys.path:
    sys.path.insert(0, "/opt/trn_rl_repo")

import numpy as np

T = 2048
C = 1024
G = 512          # per-core head-group width (8 heads x 64)
D = 64           # head dim
NH = 8           # heads per core
QCH = 512        # query chunk
KBLK = 128       # key block
AVLAG = 4        # AV deferral depth (k-blocks)


def _build_nc():
    from collections import deque
    from contextlib import ExitStack

    import concourse.bass as bass
    import concourse.mybir as mybir
    import concourse.tile as tile
    from concourse import bacc

    F32 = mybir.dt.float32
    F32R = mybir.dt.float32r
    BF16 = mybir.dt.bfloat16
    EXP = mybir.ActivationFunctionType.Exp

    nc = bacc.Bacc("TRN2", target_bir_lowering=False)

    xT = nc.dram_tensor("xT", [C, T], BF16, kind="ExternalInput")
    wq = nc.dram_tensor("wq", [C, G], BF16, kind="ExternalInput")
    wk = nc.dram_tensor("wk", [C, G], BF16, kind="ExternalInput")
    wv = nc.dram_tensor("wv", [C, G], BF16, kind="ExternalInput")
    wp = nc.dram_tensor("wp", [G, C], BF16, kind="ExternalInput")
    mask = nc.dram_tensor("mask", [128, 256], BF16, kind="ExternalInput")
    out = nc.dram_tensor("out", [T, C], BF16, kind="ExternalOutput")

    with tile.TileContext(nc) as tc, ExitStack() as ctx:
        persist = ctx.enter_context(tc.tile_pool(name="persist", bufs=1))
        xw = ctx.enter_context(tc.tile_pool(name="xw", bufs=1))
        wsl = ctx.enter_context(tc.tile_pool(name="wsl", bufs=2))
        qtkt = ctx.enter_context(tc.tile_pool(name="qtkt", bufs=2))
        ptp = ctx.enter_context(tc.tile_pool(name="ptp", bufs=7))
        nrm = ctx.enter_context(tc.tile_pool(name="nrm", bufs=2))
        osb = ctx.enter_context(tc.tile_pool(name="osb", bufs=2))
        wpp = ctx.enter_context(tc.tile_pool(name="wpp", bufs=1))
        pss = ctx.enter_context(tc.tile_pool(name="pss", bufs=2, space="PSUM"))
        psy = ctx.enter_context(tc.tile_pool(name="psy", bufs=1, space="PSUM"))
        pfl = ctx.enter_context(tc.tile_pool(name="pfl", bufs=2, space="PSUM"))

        VA = [persist.tile([128, NH * 128], BF16, name=f"va{i}", tag=f"va{i}")
              for i in range(16)]
        YT = [persist.tile([128, T], BF16, name=f"yt{i}", tag=f"yt{i}")
              for i in range(4)]
        MSK = persist.tile([128, 256], BF16, name="msk", tag="msk")
        ones_f32 = persist.tile([128, 64], F32, name="ones_f32", tag="ones_f32")
        # 65-deep contraction selection matrix: row 0 = e(0:64), row 64 =
        # e(64:128), rows 1..63 zero (single-partition writes must land on
        # partition bases 0/64, so the two 1/l rows live at 0 and 64)
        SEL = persist.tile([65, 128], BF16, name="sel", tag="sel")
        sel_f32 = persist.tile([65, 128], F32, name="sel_f32", tag="sel_f32")
        LR2 = [
            persist.tile([65, 512], BF16, name=f"lr2{i}", tag=f"lr2{i}")
            for i in range(2)
        ]

        # ---- coarse input DMA, issued across four engine queues ----
        XTall = xw.tile([128, 8 * T], BF16, name="xall", tag="xall")
        WVall = wsl.tile([128, 8 * G], BF16, name="wvall", tag="wvall")
        WPall = wpp.tile([128, 4 * C], BF16, name="wpall", tag="wpall")

        xt_s = XTall.rearrange("p (c t) -> p c t", t=T)
        xt_d = xT.rearrange("(c p) t -> p c t", p=128)
        wv_s = WVall.rearrange("p (c g) -> p c g", g=G)
        wv_d = wv.rearrange("(c p) g -> p c g", p=128)

        # ALL input DMAs ride the sync engine's single hardware queue in
        # strict need-order: one queue transfers in order at ~330 GB/s, so
        # in-order issue IS the bandwidth prioritization.  (Spreading across
        # engines was tried: concurrent queues share the same ~330 GB/s and
        # late-needed bulk blocks starved the critical first x columns.)
        nc.sync.dma_start(out=wv_s[:, 0:1, :], in_=wv_d[:, 0:1, :])
        nc.sync.dma_start(out=xt_s[:, 0:4, 0:128], in_=xt_d[:, 0:4, 0:128])
        nc.sync.dma_start(out=wv_s[:, 1:4, :], in_=wv_d[:, 1:4, :])
        nc.sync.dma_start(out=xt_s[:, 4:8, 0:128], in_=xt_d[:, 4:8, 0:128])
        nc.sync.dma_start(out=wv_s[:, 4:8, :], in_=wv_d[:, 4:8, :])
        nc.sync.dma_start(out=xt_s[:, :, 128:512], in_=xt_d[:, :, 128:512])
        nc.sync.dma_start(
            out=xt_s[:, :, 512:1024], in_=xt_d[:, :, 512:1024]
        )
        nc.sync.dma_start(
            out=xt_s[:, :, 1024:1536], in_=xt_d[:, :, 1024:1536]
        )
        nc.sync.dma_start(
            out=xt_s[:, :, 1536:2048], in_=xt_d[:, :, 1536:2048]
        )
        nc.sync.dma_start(out=MSK, in_=mask[:, :])

        def XTc(c, a, b):
            return XTall[:, c * T + a : c * T + b]

        nc.vector.memset(ones_f32, 1.0)
        # selection matrix for the combined two-head norm broadcast matmul
        nc.vector.memset(sel_f32, 0.0)
        nc.vector.tensor_copy(sel_f32[0:1, 0:64], ones_f32[0:1, 0:64])
        nc.vector.tensor_copy(sel_f32[64:65, 64:128], ones_f32[0:1, 0:64])
        nc.vector.tensor_copy(SEL, sel_f32)
        nc.vector.memset(LR2[0], 0.0)
        nc.vector.memset(LR2[1], 0.0)

        # V-augmentation ones columns
        ones_col = ones_f32[:, 0:8].rearrange("p (h o) -> p h o", o=1)
        for tb in range(16):
            vdst = VA[tb].rearrange("p (h e) -> p h e", e=128)[:, :, 64:65]
            nc.vector.tensor_copy(vdst, ones_col)

        # ---------------- phase 0: V ----------------
        def emit_v_units():
            for tb in range(16):
                ps = pfl.tile([128, 512], F32, name="fill", tag="fill")
                for c in range(8):
                    nc.tensor.matmul(
                        ps,
                        XTc(c, tb * 128, (tb + 1) * 128),
                        WVall[:, c * G : (c + 1) * G],
                        start=(c == 0),
                        stop=(c == 7),
                    )
                vdst = VA[tb].rearrange("p (h e) -> p h e", e=128)[:, :, 0:64]
                nc.vector.tensor_copy(
                    vdst, ps.rearrange("p (h d) -> p h d", d=64)
                )

        # ---------------- QK machinery ----------------
        def emit_w_slices(hp, engine_q, engine_k):
            wqh = wsl.tile([128, 8 * 128], BF16, name="wqh", tag="wqh")
            wkh = wsl.tile([128, 8 * 128], BF16, name="wkh", tag="wkh")
            engine_q.dma_start(
                out=wqh.rearrange("p (c h) -> p c h", h=128),
                in_=wq.rearrange("(c p) g -> p c g", p=128)[
                    :, :, hp * 128 : (hp + 1) * 128
                ],
            )
            engine_k.dma_start(
                out=wkh.rearrange("p (c h) -> p c h", h=128),
                in_=wk.rearrange("(c p) g -> p c g", p=128)[
                    :, :, hp * 128 : (hp + 1) * 128
                ],
            )
            return {"q": wqh, "k": wkh}

        def make_qk_units(hp, wtiles):
            """QK projection split into half-units (4 matmuls each) for
            fine-grained filler pacing."""
            qt = qtkt.tile([128, T], BF16, name="qtP", tag="qtP")
            kt = qtkt.tile([128, T], BF16, name="ktP", tag="ktP")
            units = []
            for t4 in range(4):
                for mat, dst in (("q", qt), ("k", kt)):
                    box = {}
                    wt = wtiles[mat]

                    def unit_a(wt=wt, t4=t4, box=box):
                        ps = pfl.tile([128, 512], F32, name="fill", tag="fill")
                        box["ps"] = ps
                        for c in range(4):
                            nc.tensor.matmul(
                                ps,
                                wt[:, c * 128 : (c + 1) * 128],
                                XTc(c, t4 * 512, (t4 + 1) * 512),
                                start=(c == 0),
                                stop=False,
                            )

                    def unit_b(wt=wt, dst=dst, t4=t4, box=box):
                        ps = box["ps"]
                        for c in range(4, 8):
                            nc.tensor.matmul(
                                ps,
                                wt[:, c * 128 : (c + 1) * 128],
                                XTc(c, t4 * 512, (t4 + 1) * 512),
                                start=False,
                                stop=(c == 7),
                            )
                        nc.vector.tensor_copy(
                            dst[:, t4 * 512 : (t4 + 1) * 512], ps
                        )

                    units.append(unit_a)
                    units.append(unit_b)
            return qt, kt, units

        # ---------- proj units (tail / fillers for pair 3) ----------
        def proj_units(tb):
            ot = {}
            def unit_ch(ch):
                def unit():
                    if ch == 0:
                        ot["t"] = osb.tile([128, C], BF16, name="ot", tag="ot")
                    ps = pfl.tile([128, 512], F32, name="fill", tag="fill")
                    for cb in range(4):
                        nc.tensor.matmul(
                            ps,
                            YT[cb][:, tb * 128 : (tb + 1) * 128],
                            WPall[:, cb * C + ch * 512 : cb * C + (ch + 1) * 512],
                            start=(cb == 0),
                            stop=(cb == 3),
                        )
                    nc.vector.tensor_copy(
                        ot["t"][:, ch * 512 : (ch + 1) * 512], ps
                    )
                    nc.sync.dma_start(
                        out=out[
                            tb * 128 : (tb + 1) * 128,
                            ch * 512 : (ch + 1) * 512,
                        ],
                        in_=ot["t"][:, ch * 512 : (ch + 1) * 512],
                    )
                return unit
            return [unit_ch(0), unit_ch(1)]

        def tail_units(qc):
            units = []
            for tb in range(qc * 4, qc * 4 + 4):
                units.extend(proj_units(tb))
            return units

        # ---------------- attention ----------------
        fill_q = deque()
        credit = [0.0]
        remkb = [40]

        def pump(n):
            for _ in range(min(n, len(fill_q))):
                fill_q.popleft()()

        def pump_paced():
            # adaptive: spread the current queue over the k-blocks left in
            # this head-pair so the PE never starves near the hp boundary
            if remkb[0] > 0:
                credit[0] += len(fill_q) / remkb[0]
                remkb[0] -= 1
            n = int(credit[0])
            if n > 0:
                n = min(n, len(fill_q))
                credit[0] -= n
                pump(n)

        def attention(hp, qt, kt, qc):
            q0 = qc * QCH
            nkb = (qc + 1) * 4
            hA, hB = 2 * hp, 2 * hp + 1
            ytA = psy.tile([128, QCH], F32, name="ytA", tag="ytA")
            ytB = psy.tile([128, QCH], F32, name="ytB", tag="ytB")

            def emit_av(idx, pt, off, w, kb):
                # each head's AV split into two column-band tiles (full
                # K=128, out dims 0:63 / 64:127) so the pair co-issues and
                # dual-streams the PE like the S pairs; the two halves hit
                # disjoint PSUM partition ranges (no accumulation race)
                st = idx == 0
                sp = idx == nkb - 1
                for h, ytX, pof in ((hA, ytA, 0), (hB, ytB, 512)):
                    pcol = pt[:, pof + off : pof + off + w]
                    nc.tensor.matmul(
                        ytX[0:64, off : off + w],
                        VA[kb][:, h * 128 : h * 128 + 64],
                        pcol,
                        start=st,
                        stop=sp,
                        tile_position=(0, 0),
                    )
                    nc.tensor.matmul(
                        ytX[64:128, off : off + w],
                        VA[kb][:, h * 128 + 64 : h * 128 + 128],
                        pcol,
                        start=st,
                        stop=sp,
                        tile_position=(0, 64),
                    )

            pend = deque()
            # diagonal (masked) blocks first: their gpsimd mask-muls finish
            # early, so the end-of-qc AV flush never waits on the mask
            kb_order = list(range(qc * 4, nkb)) + list(range(0, qc * 4))
            for idx, kb in enumerate(kb_order):
                j = kb - qc * 4
                off = j * 128 if j >= 1 else 0
                w = 512 - off
                ksl = slice(kb * KBLK, (kb + 1) * KBLK)
                sAB = pss.tile([128, 1024], F32, name="sAB", tag="sAB")
                nc.tensor.matmul(
                    sAB[:, off : 512],
                    kt[0:64, ksl],
                    qt[0:64, q0 + off : q0 + QCH],
                    start=True,
                    stop=True,
                    tile_position=(0, 0),
                )
                nc.tensor.matmul(
                    sAB[:, 512 + off : 1024],
                    kt[64:128, ksl],
                    qt[64:128, q0 + off : q0 + QCH],
                    start=True,
                    stop=True,
                    tile_position=(64, 0),
                )
                pt = ptp.tile([128, 1024], BF16, name="pt", tag="pt")
                nc.scalar.activation(
                    pt[:, off:1024], sAB[:, off:1024], EXP, scale=0.125
                )
                if j >= 0:
                    pv = pt.rearrange("p (s q) -> p s q", s=2)[
                        :, :, off : off + 128
                    ]
                    nc.gpsimd.tensor_mul(
                        pv, pv, MSK.rearrange("p (s q) -> p s q", s=2)
                    )
                pump_paced()
                if len(pend) == AVLAG:
                    emit_av(*pend.popleft())
                pend.append((idx, pt, off, w, kb))
            while pend:
                emit_av(*pend.popleft())
                # keep the PE fed while the flush AVs wait on the last exps
                if len(pend) % 2 == 1:
                    pump(1)
            lr2 = LR2[(hp * 4 + qc) % 2]
            yslA = YT[hp][0:64, q0 : q0 + QCH]
            yslB = YT[hp][64:128, q0 : q0 + QCH]
            for sub, (yt, ysl) in enumerate(((ytA, yslA), (ytB, yslB))):
                nc.vector.tensor_copy(ysl, yt[0:64, :])
                lf = nrm.tile([1, 512], F32, name="lf", tag="lf")
                nc.vector.tensor_copy(lf, yt[64:65, :])
                lf2 = nrm.tile([1, 512], F32, name="lf2", tag="lf2")
                nc.vector.reciprocal_approx_fast(lf2, lf)
                nc.vector.tensor_copy(lr2[sub * 64 : sub * 64 + 1, :], lf2)

            def norm_fin(yslA=yslA, yslB=yslB, lr2=lr2):
                rb = pfl.tile([128, 512], F32, name="fill", tag="fill")
                nc.tensor.matmul(rb, SEL, lr2, start=True, stop=True)
                nc.vector.tensor_mul(yslA, yslA, rb[0:64, :])
                nc.vector.tensor_mul(yslB, yslB, rb[64:128, :])

            fill_q.appendleft(norm_fin)

        # ---------------- main schedule ----------------
        wtiles0 = emit_w_slices(0, nc.sync, nc.sync)
        nc.sync.dma_start(
            out=WPall.rearrange("p (b c) -> p b c", c=C),
            in_=wp.rearrange("(b p) c -> p b c", p=128),
        )
        emit_v_units()
        qt, kt, units = make_qk_units(0, wtiles0)
        for u in units:
            u()
        for hp in range(4):
            nqt = nkt = None
            remkb[0] = 40
            credit[0] = 0.0
            if hp < 3:
                nwt = emit_w_slices(hp + 1, nc.sync, nc.sync)
                nqt, nkt, nunits = make_qk_units(hp + 1, nwt)
                fill_q.extend(nunits)
            for qc in (3, 2, 1, 0):
                attention(hp, qt, kt, qc)
                if hp == 3 and qc >= 1:
                    fill_q.extend(tail_units(qc))
            pump(len(fill_q))
            if hp < 3:
                qt, kt = nqt, nkt
        for u in tail_units(0):
            u()

    nc.compile()
    return nc


_NC_CACHE = None


def kernel(x0, w_attn, w_proj, _trace=False, _tmpdir=None):
    global _NC_CACHE
    import ml_dtypes

    from concourse.bass_utils import run_bass_kernel_spmd

    BF = ml_dtypes.bfloat16
    x0 = np.asarray(x0, dtype=np.float32)
    w_attn = np.asarray(w_attn, dtype=np.float32)
    w_proj = np.asarray(w_proj, dtype=np.float32)
    B = x0.shape[0]

    if _NC_CACHE is None:
        _NC_CACHE = _build_nc()
    nc = _NC_CACHE

    tri = np.triu(np.ones((128, 128), dtype=np.float32))
    msk = np.concatenate([tri, tri], axis=1).astype(BF)
    in_maps = []
    for core in range(8):
        b, g = divmod(core, 2)
        in_maps.append(
            {
                "xT": np.ascontiguousarray(x0[b].T).astype(BF),
                "wq": np.ascontiguousarray(
                    w_attn[:, g * G : (g + 1) * G]
                ).astype(BF),
                "wk": np.ascontiguousarray(
                    w_attn[:, C + g * G : C + (g + 1) * G]
                ).astype(BF),
                "wv": np.ascontiguousarray(
                    w_attn[:, 2 * C + g * G : 2 * C + (g + 1) * G]
                ).astype(BF),
                "wp": np.ascontiguousarray(
                    w_proj[g * G : (g + 1) * G, :]
                ).astype(BF),
                "mask": msk,
            }
        )

    res = run_bass_kernel_spmd(
        nc, in_maps, list(range(8)), trace=_trace, tmpdir=_tmpdir
    )
    outp = np.empty((B, T, C), dtype=np.float32)
    for b in range(B):
        outp[b] = res.results[2 * b]["out"].astype(np.float32) + res.results[
            2 * b + 1
        ]["out"].astype(np.float32)
    if _trace:
        kernel.last_exec_time_ns = res.exec_time_ns
    return outp


# revision 27
# speedup vs baseline: 1.0480x; 1.0480x over previous
"""Causal self-attention (B=4, T=2048, C=1024, H=16) on 8 trn2 NeuronCores.

Sharding: core = (batch b, head-group g), b in 0..3, g in 0..1. Each core does
8 heads of one batch element (Megatron column split of w_attn, row split of
w_proj); host sums the two partial projection outputs per batch element.

Per-core kernel, v4 (reduced PE work + coarse DMA):
 - All DRAM inputs bf16 (host casts); attention matmul operands bf16.
 - Coarse multi-c-block DMA descriptors issued in parallel from four engine
   queues at start (the v3 per-chunk DMAs serialized ~600ns/issue on sync and
   starved the V phase).
 - Q^T,K^T computed transposed (lhsT=W-block, rhs=x^T-block) so attention
   needs no transposes; V natural with a ones column per head so the
   attention AV matmul accumulates the softmax denominator l for free.
 - Attention per head-pair: S^T for both heads row-tiled into one
   [128,1024] PSUM tile per k-block; one exp (scale=1/8 folded in, no
   max-subtraction -- scores are N(0,1)); causal mask only on diagonal
   blocks via one doubled-mask bf16 multiply ON GPSIMD; AV deferred four
   k-blocks so exp latency hides; filler units (next head-pair's QK
   projection, output projection) credit-paced into every k-block.
 - qc processed descending (3,2,1,0): the long qc pipelines come first and
   the tiny qc0 flush lands where fillers still exist; hp3's projection
   tail units enqueue one qc earlier.
 - Normalization: l rows copied off PSUM + reciprocal on DVE, then ONE
   fp32r matmul per qc (contraction-2 selection matrix) builds the [128,512]
   broadcast of 1/l for BOTH heads (v3 used two rank-1 matmuls), and the Y
   drain is fused with the 1/l multiply in a single DVE tensor_tensor op
   per head (replaces v3's separate copy + in-place multiply).  The norm
   unit is pushed to the FRONT of the filler queue so the Y PSUM frees
   within a k-block or two.  (partition_broadcast was tried and reverted:
   it lives in a different gpsimd ucode library than tensor_tensor, and
   the resulting per-qc library swaps cost ~14us each.)
"""

import sys

if "/opt/trn_rl_repo" not in sys.path:
    sys.path.insert(0, "/opt/trn_rl_repo")

import numpy as np

T = 2048
C = 1024
G = 512          # per-core head-group width (8 heads x 64)
D = 64           # head dim
NH = 8           # heads per core
QCH = 512        # query chunk
KBLK = 128       # key block
AVLAG = 5        # AV deferral depth (k-blocks)


def _build_nc():
    from collections import deque
    from contextlib import ExitStack

    import concourse.bass as bass
    import concourse.mybir as mybir
    import concourse.tile as tile
    from concourse import bacc

    F32 = mybir.dt.float32
    F32R = mybir.dt.float32r
    BF16 = mybir.dt.bfloat16
    EXP = mybir.ActivationFunctionType.Exp

    nc = bacc.Bacc("TRN2", target_bir_lowering=False)

    xT = nc.dram_tensor("xT", [C, T], BF16, kind="ExternalInput")
    wq = nc.dram_tensor("wq", [C, G], BF16, kind="ExternalInput")
    wk = nc.dram_tensor("wk", [C, G], BF16, kind="ExternalInput")
    wv = nc.dram_tensor("wv", [C, G], BF16, kind="ExternalInput")
    wp = nc.dram_tensor("wp", [G, C], BF16, kind="ExternalInput")
    mask = nc.dram_tensor("mask", [128, 256], BF16, kind="ExternalInput")
    out = nc.dram_tensor("out", [T, C], BF16, kind="ExternalOutput")

    with tile.TileContext(nc) as tc, ExitStack() as ctx:
        persist = ctx.enter_context(tc.tile_pool(name="persist", bufs=1))
        xw = ctx.enter_context(tc.tile_pool(name="xw", bufs=1))
        wsl = ctx.enter_context(tc.tile_pool(name="wsl", bufs=2))
        qtkt = ctx.enter_context(tc.tile_pool(name="qtkt", bufs=2))
        ptp = ctx.enter_context(tc.tile_pool(name="ptp", bufs=8))
        nrm = ctx.enter_context(tc.tile_pool(name="nrm", bufs=2))
        osb = ctx.enter_context(tc.tile_pool(name="osb", bufs=2))
        wpp = ctx.enter_context(tc.tile_pool(name="wpp", bufs=1))
        pss = ctx.enter_context(tc.tile_pool(name="pss", bufs=2, space="PSUM"))
        psy = ctx.enter_context(tc.tile_pool(name="psy", bufs=1, space="PSUM"))
        pfl = ctx.enter_context(tc.tile_pool(name="pfl", bufs=2, space="PSUM"))

        VA = [persist.tile([128, NH * 128], BF16, name=f"va{i}", tag=f"va{i}")
              for i in range(16)]
        YT = [persist.tile([128, T], BF16, name=f"yt{i}", tag=f"yt{i}")
              for i in range(4)]
        MSK = persist.tile([128, 256], BF16, name="msk", tag="msk")
        ones_f32 = persist.tile([128, 64], F32, name="ones_f32", tag="ones_f32")
        # 65-deep contraction selection matrix: row 0 = e(0:64), row 64 =
        # e(64:128), rows 1..63 zero (single-partition writes must land on
        # partition bases 0/64, so the two 1/l rows live at 0 and 64)
        SEL = persist.tile([65, 128], BF16, name="sel", tag="sel")
        sel_f32 = persist.tile([65, 128], F32, name="sel_f32", tag="sel_f32")
        LR2 = [
            persist.tile([65, 512], BF16, name=f"lr2{i}", tag=f"lr2{i}")
            for i in range(2)
        ]

        # ---- coarse input DMA, issued across four engine queues ----
        XTall = xw.tile([128, 8 * T], BF16, name="xall", tag="xall")
        WVall = wsl.tile([128, 8 * G], BF16, name="wvall", tag="wvall")
        WPall = wpp.tile([128, 4 * C], BF16, name="wpall", tag="wpall")

        xt_s = XTall.rearrange("p (c t) -> p c t", t=T)
        xt_d = xT.rearrange("(c p) t -> p c t", p=128)
        wv_s = WVall.rearrange("p (c g) -> p c g", g=G)
        wv_d = wv.rearrange("(c p) g -> p c g", p=128)

        # ALL input DMAs ride the sync engine's single hardware queue in
        # strict need-order: one queue transfers in order at ~330 GB/s, so
        # in-order issue IS the bandwidth prioritization.  (Spreading across
        # engines was tried: concurrent queues share the same ~330 GB/s and
        # late-needed bulk blocks starved the critical first x columns.)
        nc.sync.dma_start(out=wv_s[:, 0:1, :], in_=wv_d[:, 0:1, :])
        nc.sync.dma_start(out=xt_s[:, 0:4, 0:128], in_=xt_d[:, 0:4, 0:128])
        nc.sync.dma_start(out=wv_s[:, 1:4, :], in_=wv_d[:, 1:4, :])
        nc.sync.dma_start(out=xt_s[:, 4:8, 0:128], in_=xt_d[:, 4:8, 0:128])
        nc.sync.dma_start(out=wv_s[:, 4:8, :], in_=wv_d[:, 4:8, :])
        nc.sync.dma_start(out=xt_s[:, :, 128:512], in_=xt_d[:, :, 128:512])
        nc.sync.dma_start(
            out=xt_s[:, :, 512:1024], in_=xt_d[:, :, 512:1024]
        )
        nc.sync.dma_start(
            out=xt_s[:, :, 1024:1536], in_=xt_d[:, :, 1024:1536]
        )
        nc.sync.dma_start(
            out=xt_s[:, :, 1536:2048], in_=xt_d[:, :, 1536:2048]
        )
        nc.sync.dma_start(out=MSK, in_=mask[:, :])

        def XTc(c, a, b):
            return XTall[:, c * T + a : c * T + b]

        nc.vector.memset(ones_f32, 1.0)
        # selection matrix for the combined two-head norm broadcast matmul
        nc.vector.memset(sel_f32, 0.0)
        nc.vector.tensor_copy(sel_f32[0:1, 0:64], ones_f32[0:1, 0:64])
        nc.vector.tensor_copy(sel_f32[64:65, 64:128], ones_f32[0:1, 0:64])
        nc.vector.tensor_copy(SEL, sel_f32)
        nc.vector.memset(LR2[0], 0.0)
        nc.vector.memset(LR2[1], 0.0)

        # V-augmentation ones columns
        ones_col = ones_f32[:, 0:8].rearrange("p (h o) -> p h o", o=1)
        for tb in range(16):
            vdst = VA[tb].rearrange("p (h e) -> p h e", e=128)[:, :, 64:65]
            nc.vector.tensor_copy(vdst, ones_col)

        # ---------------- phase 0: V ----------------
        v_done = [0]

        def make_v_units():
            units = []

            def unit_tb(tb):
                def unit():
                    ps = pfl.tile([128, 512], F32, name="fill", tag="fill")
                    for c in range(8):
                        nc.tensor.matmul(
                            ps,
                            XTc(c, tb * 128, (tb + 1) * 128),
                            WVall[:, c * G : (c + 1) * G],
                            start=(c == 0),
                            stop=(c == 7),
                        )
                    vdst = VA[tb].rearrange("p (h e) -> p h e", e=128)[
                        :, :, 0:64
                    ]
                    nc.vector.tensor_copy(
                        vdst, ps.rearrange("p (h d) -> p h d", d=64)
                    )
                    v_done[0] += 1
                return unit

            for tb in range(16):
                units.append(unit_tb(tb))
            return units

        # ---------------- QK machinery ----------------
        def emit_w_slices(hp, engine_q, engine_k):
            wqh = wsl.tile([128, 8 * 128], BF16, name="wqh", tag="wqh")
            wkh = wsl.tile([128, 8 * 128], BF16, name="wkh", tag="wkh")
            engine_q.dma_start(
                out=wqh.rearrange("p (c h) -> p c h", h=128),
                in_=wq.rearrange("(c p) g -> p c g", p=128)[
                    :, :, hp * 128 : (hp + 1) * 128
                ],
            )
            engine_k.dma_start(
                out=wkh.rearrange("p (c h) -> p c h", h=128),
                in_=wk.rearrange("(c p) g -> p c g", p=128)[
                    :, :, hp * 128 : (hp + 1) * 128
                ],
            )
            return {"q": wqh, "k": wkh}

        def make_qk_units(hp, wtiles):
            """QK projection split into half-units (4 matmuls each) for
            fine-grained filler pacing."""
            qt = qtkt.tile([128, T], BF16, name="qtP", tag="qtP")
            kt = qtkt.tile([128, T], BF16, name="ktP", tag="ktP")
            units = []
            for t4 in range(4):
                for mat, dst in (("q", qt), ("k", kt)):
                    box = {}
                    wt = wtiles[mat]

                    def unit_a(wt=wt, t4=t4, box=box):
                        ps = pfl.tile([128, 512], F32, name="fill", tag="fill")
                        box["ps"] = ps
                        for c in range(4):
                            nc.tensor.matmul(
                                ps,
                                wt[:, c * 128 : (c + 1) * 128],
                                XTc(c, t4 * 512, (t4 + 1) * 512),
                                start=(c == 0),
                                stop=False,
                            )

                    def unit_b(wt=wt, dst=dst, t4=t4, box=box):
                        ps = box["ps"]
                        for c in range(4, 8):
                            nc.tensor.matmul(
                                ps,
                                wt[:, c * 128 : (c + 1) * 128],
                                XTc(c, t4 * 512, (t4 + 1) * 512),
                                start=False,
                                stop=(c == 7),
                            )
                        nc.vector.tensor_copy(
                            dst[:, t4 * 512 : (t4 + 1) * 512], ps
                        )

                    units.append(unit_a)
                    units.append(unit_b)
            return qt, kt, units

        # ---------- proj units (tail / fillers for pair 3) ----------
        def proj_units(tb):
            ot = {}
            def unit_ch(ch):
                def unit():
                    if ch == 0:
                        ot["t"] = osb.tile([128, C], BF16, name="ot", tag="ot")
                    ps = pfl.tile([128, 512], F32, name="fill", tag="fill")
                    for cb in range(4):
                        nc.tensor.matmul(
                            ps,
                            YT[cb][:, tb * 128 : (tb + 1) * 128],
                            WPall[:, cb * C + ch * 512 : cb * C + (ch + 1) * 512],
                            start=(cb == 0),
                            stop=(cb == 3),
                        )
                    nc.vector.tensor_copy(
                        ot["t"][:, ch * 512 : (ch + 1) * 512], ps
                    )
                    nc.sync.dma_start(
                        out=out[
                            tb * 128 : (tb + 1) * 128,
                            ch * 512 : (ch + 1) * 512,
                        ],
                        in_=ot["t"][:, ch * 512 : (ch + 1) * 512],
                    )
                return unit
            return [unit_ch(0), unit_ch(1)]

        def tail_units(qc):
            units = []
            for tb in range(qc * 4, qc * 4 + 4):
                units.extend(proj_units(tb))
            return units

        # ---------------- attention ----------------
        fill_q = deque()
        credit = [0.0]
        remkb = [40]

        def pump(n):
            for _ in range(min(n, len(fill_q))):
                fill_q.popleft()()

        def pump_paced():
            # adaptive: spread the current queue over the k-blocks left in
            # this head-pair so the PE never starves near the hp boundary
            if remkb[0] > 0:
                credit[0] += len(fill_q) / remkb[0]
                remkb[0] -= 1
            n = int(credit[0])
            if n > 0:
                n = min(n, len(fill_q))
                credit[0] -= n
                pump(n)

        def attention(hp, qt, kt, qc):
            q0 = qc * QCH
            nkb = (qc + 1) * 4
            hA, hB = 2 * hp, 2 * hp + 1
            ytA = psy.tile([128, QCH], F32, name="ytA", tag="ytA")
            ytB = psy.tile([128, QCH], F32, name="ytB", tag="ytB")

            def emit_av(idx, pt, off, w, kb):
                nc.tensor.matmul(
                    ytA[:, off : off + w],
                    VA[kb][:, hA * 128 : hA * 128 + 128],
                    pt[:, off : off + w],
                    start=(idx == 0),
                    stop=(idx == nkb - 1),
                )
                nc.tensor.matmul(
                    ytB[:, off : off + w],
                    VA[kb][:, hB * 128 : hB * 128 + 128],
                    pt[:, 512 + off : 512 + off + w],
                    start=(idx == 0),
                    stop=(idx == nkb - 1),
                )

            pend = deque()
            # absorb the qc-start scalar backlog (previous qc's flush exps)
            pump(2)
            # diagonal (masked) blocks first: their gpsimd mask-muls finish
            # early, so the end-of-qc AV flush never waits on the mask
            kb_order = list(range(qc * 4, nkb)) + list(range(0, qc * 4))
            for idx, kb in enumerate(kb_order):
                j = kb - qc * 4
                off = j * 128 if j >= 1 else 0
                w = 512 - off
                ksl = slice(kb * KBLK, (kb + 1) * KBLK)
                sAB = pss.tile([128, 1024], F32, name="sAB", tag="sAB")
                nc.tensor.matmul(
                    sAB[:, off : 512],
                    kt[0:64, ksl],
                    qt[0:64, q0 + off : q0 + QCH],
                    start=True,
                    stop=True,
                    tile_position=(0, 0),
                )
                nc.tensor.matmul(
                    sAB[:, 512 + off : 1024],
                    kt[64:128, ksl],
                    qt[64:128, q0 + off : q0 + QCH],
                    start=True,
                    stop=True,
                    tile_position=(64, 0),
                )
                pt = ptp.tile([128, 1024], BF16, name="pt", tag="pt")
                nc.scalar.activation(
                    pt[:, off:1024], sAB[:, off:1024], EXP, scale=0.125
                )
                if j >= 0:
                    pv = pt.rearrange("p (s q) -> p s q", s=2)[
                        :, :, off : off + 128
                    ]
                    nc.gpsimd.tensor_mul(
                        pv, pv, MSK.rearrange("p (s q) -> p s q", s=2)
                    )
                pump_paced()
                if len(pend) == AVLAG:
                    emit_av(*pend.popleft())
                pend.append((idx, pt, off, w, kb))
            while pend:
                emit_av(*pend.popleft())
                # keep the PE fed while the flush AVs wait on the last exps
                if len(pend) % 2 == 1:
                    pump(1)
            lr2 = LR2[(hp * 4 + qc) % 2]
            yslA = YT[hp][0:64, q0 : q0 + QCH]
            yslB = YT[hp][64:128, q0 : q0 + QCH]
            for sub, (yt, ysl) in enumerate(((ytA, yslA), (ytB, yslB))):
                nc.vector.tensor_copy(ysl, yt[0:64, :])
                lf = nrm.tile([1, 512], F32, name="lf", tag="lf")
                nc.vector.tensor_copy(lf, yt[64:65, :])
                lf2 = nrm.tile([1, 512], F32, name="lf2", tag="lf2")
                nc.vector.reciprocal_approx_fast(lf2, lf)
                nc.vector.tensor_copy(lr2[sub * 64 : sub * 64 + 1, :], lf2)

            def norm_fin(yslA=yslA, yslB=yslB, lr2=lr2):
                rb = pfl.tile([128, 512], F32, name="fill", tag="fill")
                nc.tensor.matmul(rb, SEL, lr2, start=True, stop=True)
                nc.vector.tensor_mul(yslA, yslA, rb[0:64, :])
                nc.vector.tensor_mul(yslB, yslB, rb[64:128, :])

            fill_q.appendleft(norm_fin)

        # ---------------- main schedule ----------------
        wtiles0 = emit_w_slices(0, nc.sync, nc.sync)
        nc.sync.dma_start(
            out=WPall.rearrange("p (b c) -> p b c", c=C),
            in_=wp.rearrange("(b p) c -> p b c", p=128),
        )
        # only the first 4 V blocks run inline; the rest become attention
        # fillers for hp0 (ascending qc order so VA deadlines stay loose),
        # letting attention start ~20us earlier and filling hp0's stalls
        # with dense V work
        v_units = make_v_units()
        for u in v_units[0:4]:
            u()
        qt, kt, units = make_qk_units(0, wtiles0)
        for u in units:
            u()
        for hp in range(4):
            nqt = nkt = None
            remkb[0] = 40
            credit[0] = 0.0
            if hp == 0:
                fill_q.extend(v_units[4:])
            if hp < 3:
                nwt = emit_w_slices(hp + 1, nc.sync, nc.sync)
                nqt, nkt, nunits = make_qk_units(hp + 1, nwt)
                fill_q.extend(nunits)
            qc_iter = (0, 1, 2, 3) if hp == 0 else (3, 2, 1, 0)
            for qc in qc_iter:
                if hp == 0:
                    # force the V prefix this qc's AV blocks will read
                    while v_done[0] < min(16, (qc + 1) * 4):
                        fill_q.popleft()()
                attention(hp, qt, kt, qc)
                if hp == 3 and qc >= 1:
                    fill_q.extend(tail_units(qc))
            pump(len(fill_q))
            if hp < 3:
                qt, kt = nqt, nkt
        for u in tail_units(0):
            u()

    nc.compile()
    return nc


_NC_CACHE = None


def kernel(x0, w_attn, w_proj, _trace=False, _tmpdir=None):
    global _NC_CACHE
    import ml_dtypes

    from concourse.bass_utils import run_bass_kernel_spmd

    BF = ml_dtypes.bfloat16
    x0 = np.asarray(x0, dtype=np.float32)
    w_attn = np.asarray(w_attn, dtype=np.float32)
    w_proj = np.asarray(w_proj, dtype=np.float32)
    B = x0.shape[0]

    if _NC_CACHE is None:
        _NC_CACHE = _build_nc()
    nc = _NC_CACHE

    tri = np.triu(np.ones((128, 128), dtype=np.float32))
    msk = np.concatenate([tri, tri], axis=1).astype(BF)
    in_maps = []
    for core in range(8):
        b, g = divmod(core, 2)
        in_maps.append(
            {
                "xT": np.ascontiguousarray(x0[b].T).astype(BF),
                "wq": np.ascontiguousarray(
                    w_attn[:, g * G : (g + 1) * G]
                ).astype(BF),
                "wk": np.ascontiguousarray(
                    w_attn[:, C + g * G : C + (g + 1) * G]
                ).astype(BF),
                "wv": np.ascontiguousarray(
                    w_attn[:, 2 * C + g * G : 2 * C + (g + 1) * G]
                ).astype(BF),
                "wp": np.ascontiguousarray(
                    w_proj[g * G : (g + 1) * G, :]
                ).astype(BF),
                "mask": msk,
            }
        )

    res = run_bass_kernel_spmd(
        nc, in_maps, list(range(8)), trace=_trace, tmpdir=_tmpdir
    )
    outp = np.empty((B, T, C), dtype=np.float32)
    for b in range(B):
        outp[b] = res.results[2 * b]["out"].astype(np.float32) + res.results[
            2 * b + 1
        ]["out"].astype(np.float32)
    if _trace:
        kernel.last_exec_time_ns = res.exec_time_ns
    return outp


# revision 30
# speedup vs baseline: 1.0722x; 1.0230x over previous
"""Causal self-attention (B=4, T=2048, C=1024, H=16) on 8 trn2 NeuronCores.

Sharding: core = (batch b, head-group g), b in 0..3, g in 0..1. Each core does
8 heads of one batch element (Megatron column split of w_attn, row split of
w_proj); host sums the two partial projection outputs per batch element.

Per-core kernel, v4 (reduced PE work + coarse DMA):
 - All DRAM inputs bf16 (host casts); attention matmul operands bf16.
 - Coarse multi-c-block DMA descriptors issued in parallel from four engine
   queues at start (the v3 per-chunk DMAs serialized ~600ns/issue on sync and
   starved the V phase).
 - Q^T,K^T computed transposed (lhsT=W-block, rhs=x^T-block) so attention
   needs no transposes; V natural with a ones column per head so the
   attention AV matmul accumulates the softmax denominator l for free.
 - Attention per head-pair: S^T for both heads row-tiled into one
   [128,1024] PSUM tile per k-block; one exp (scale=1/8 folded in, no
   max-subtraction -- scores are N(0,1)); causal mask only on diagonal
   blocks via one doubled-mask bf16 multiply ON GPSIMD; AV deferred four
   k-blocks so exp latency hides; filler units (next head-pair's QK
   projection, output projection) credit-paced into every k-block.
 - qc processed descending (3,2,1,0): the long qc pipelines come first and
   the tiny qc0 flush lands where fillers still exist; hp3's projection
   tail units enqueue one qc earlier.
 - Normalization: l rows copied off PSUM + reciprocal on DVE, then ONE
   fp32r matmul per qc (contraction-2 selection matrix) builds the [128,512]
   broadcast of 1/l for BOTH heads (v3 used two rank-1 matmuls), and the Y
   drain is fused with the 1/l multiply in a single DVE tensor_tensor op
   per head (replaces v3's separate copy + in-place multiply).  The norm
   unit is pushed to the FRONT of the filler queue so the Y PSUM frees
   within a k-block or two.  (partition_broadcast was tried and reverted:
   it lives in a different gpsimd ucode library than tensor_tensor, and
   the resulting per-qc library swaps cost ~14us each.)
"""

import sys

if "/opt/trn_rl_repo" not in sys.path:
    sys.path.insert(0, "/opt/trn_rl_repo")

import numpy as np

T = 2048
C = 1024
G = 512          # per-core head-group width (8 heads x 64)
D = 64           # head dim
NH = 8           # heads per core
QCH = 512        # query chunk
KBLK = 128       # key block
AVLAG = 4        # AV deferral depth (k-blocks)


def _build_nc():
    from collections import deque
    from contextlib import ExitStack

    import concourse.bass as bass
    import concourse.mybir as mybir
    import concourse.tile as tile
    from concourse import bacc

    F32 = mybir.dt.float32
    F32R = mybir.dt.float32r
    BF16 = mybir.dt.bfloat16
    EXP = mybir.ActivationFunctionType.Exp

    nc = bacc.Bacc("TRN2", target_bir_lowering=False)

    xT = nc.dram_tensor("xT", [C, T], BF16, kind="ExternalInput")
    wq = nc.dram_tensor("wq", [C, G], BF16, kind="ExternalInput")
    wk = nc.dram_tensor("wk", [C, G], BF16, kind="ExternalInput")
    wv = nc.dram_tensor("wv", [C, G], BF16, kind="ExternalInput")
    wp = nc.dram_tensor("wp", [G, C], BF16, kind="ExternalInput")
    mask = nc.dram_tensor("mask", [128, 256], BF16, kind="ExternalInput")
    out = nc.dram_tensor("out", [T, C], BF16, kind="ExternalOutput")

    with tile.TileContext(nc) as tc, ExitStack() as ctx:
        persist = ctx.enter_context(tc.tile_pool(name="persist", bufs=1))
        xw = ctx.enter_context(tc.tile_pool(name="xw", bufs=1))
        wsl = ctx.enter_context(tc.tile_pool(name="wsl", bufs=2))
        qtkt = ctx.enter_context(tc.tile_pool(name="qtkt", bufs=2))
        ptp = ctx.enter_context(tc.tile_pool(name="ptp", bufs=7))
        nrm = ctx.enter_context(tc.tile_pool(name="nrm", bufs=2))
        osb = ctx.enter_context(tc.tile_pool(name="osb", bufs=2))
        wpp = ctx.enter_context(tc.tile_pool(name="wpp", bufs=1))
        pss = ctx.enter_context(tc.tile_pool(name="pss", bufs=2, space="PSUM"))
        psy = ctx.enter_context(tc.tile_pool(name="psy", bufs=1, space="PSUM"))
        pfl = ctx.enter_context(tc.tile_pool(name="pfl", bufs=2, space="PSUM"))

        VA = [persist.tile([128, NH * 128], BF16, name=f"va{i}", tag=f"va{i}")
              for i in range(16)]
        YT = [persist.tile([128, T], BF16, name=f"yt{i}", tag=f"yt{i}")
              for i in range(4)]
        MSK = persist.tile([128, 256], BF16, name="msk", tag="msk")
        ones_f32 = persist.tile([128, 64], F32, name="ones_f32", tag="ones_f32")
        # 65-deep contraction selection matrix: row 0 = e(0:64), row 64 =
        # e(64:128), rows 1..63 zero (single-partition writes must land on
        # partition bases 0/64, so the two 1/l rows live at 0 and 64)
        SEL = persist.tile([65, 128], BF16, name="sel", tag="sel")
        sel_f32 = persist.tile([65, 128], F32, name="sel_f32", tag="sel_f32")
        LR2 = [
            persist.tile([65, 512], BF16, name=f"lr2{i}", tag=f"lr2{i}")
            for i in range(2)
        ]

        # ---- coarse input DMA, issued across four engine queues ----
        XTall = xw.tile([128, 8 * T], BF16, name="xall", tag="xall")
        WVall = wsl.tile([128, 8 * G], BF16, name="wvall", tag="wvall")
        WPall = wpp.tile([128, 4 * C], BF16, name="wpall", tag="wpall")

        xt_s = XTall.rearrange("p (c t) -> p c t", t=T)
        xt_d = xT.rearrange("(c p) t -> p c t", p=128)
        wv_s = WVall.rearrange("p (c g) -> p c g", g=G)
        wv_d = wv.rearrange("(c p) g -> p c g", p=128)

        # ALL input DMAs ride the sync engine's single hardware queue in
        # strict need-order: one queue transfers in order at ~330 GB/s, so
        # in-order issue IS the bandwidth prioritization.  (Spreading across
        # engines was tried: concurrent queues share the same ~330 GB/s and
        # late-needed bulk blocks starved the critical first x columns.)
        nc.sync.dma_start(out=wv_s[:, 0:1, :], in_=wv_d[:, 0:1, :])
        nc.sync.dma_start(out=xt_s[:, 0:4, 0:128], in_=xt_d[:, 0:4, 0:128])
        nc.sync.dma_start(out=wv_s[:, 1:4, :], in_=wv_d[:, 1:4, :])
        nc.sync.dma_start(out=xt_s[:, 4:8, 0:128], in_=xt_d[:, 4:8, 0:128])
        nc.sync.dma_start(out=wv_s[:, 4:8, :], in_=wv_d[:, 4:8, :])
        nc.sync.dma_start(out=xt_s[:, :, 128:512], in_=xt_d[:, :, 128:512])
        nc.sync.dma_start(
            out=xt_s[:, :, 512:1024], in_=xt_d[:, :, 512:1024]
        )
        nc.sync.dma_start(
            out=xt_s[:, :, 1024:1536], in_=xt_d[:, :, 1024:1536]
        )
        nc.sync.dma_start(
            out=xt_s[:, :, 1536:2048], in_=xt_d[:, :, 1536:2048]
        )
        nc.sync.dma_start(out=MSK, in_=mask[:, :])

        def XTc(c, a, b):
            return XTall[:, c * T + a : c * T + b]

        nc.vector.memset(ones_f32, 1.0)
        # selection matrix for the combined two-head norm broadcast matmul
        nc.vector.memset(sel_f32, 0.0)
        nc.vector.tensor_copy(sel_f32[0:1, 0:64], ones_f32[0:1, 0:64])
        nc.vector.tensor_copy(sel_f32[64:65, 64:128], ones_f32[0:1, 0:64])
        nc.vector.tensor_copy(SEL, sel_f32)
        nc.vector.memset(LR2[0], 0.0)
        nc.vector.memset(LR2[1], 0.0)

        # V-augmentation ones columns
        ones_col = ones_f32[:, 0:8].rearrange("p (h o) -> p h o", o=1)
        for tb in range(16):
            vdst = VA[tb].rearrange("p (h e) -> p h e", e=128)[:, :, 64:65]
            nc.vector.tensor_copy(vdst, ones_col)

        # ---------------- phase 0: V ----------------
        v_done = [0]

        def make_v_units():
            units = []

            def unit_tb(tb):
                def unit():
                    ps = pfl.tile([128, 512], F32, name="fill", tag="fill")
                    for c in range(8):
                        nc.tensor.matmul(
                            ps,
                            XTc(c, tb * 128, (tb + 1) * 128),
                            WVall[:, c * G : (c + 1) * G],
                            start=(c == 0),
                            stop=(c == 7),
                        )
                    vdst = VA[tb].rearrange("p (h e) -> p h e", e=128)[
                        :, :, 0:64
                    ]
                    nc.vector.tensor_copy(
                        vdst, ps.rearrange("p (h d) -> p h d", d=64)
                    )
                    v_done[0] += 1
                return unit

            for tb in range(16):
                units.append(unit_tb(tb))
            return units

        # ---------------- QK machinery ----------------
        def emit_w_slices(hp, engine_q, engine_k):
            wqh = wsl.tile([128, 8 * 128], BF16, name="wqh", tag="wqh")
            wkh = wsl.tile([128, 8 * 128], BF16, name="wkh", tag="wkh")
            engine_q.dma_start(
                out=wqh.rearrange("p (c h) -> p c h", h=128),
                in_=wq.rearrange("(c p) g -> p c g", p=128)[
                    :, :, hp * 128 : (hp + 1) * 128
                ],
            )
            engine_k.dma_start(
                out=wkh.rearrange("p (c h) -> p c h", h=128),
                in_=wk.rearrange("(c p) g -> p c g", p=128)[
                    :, :, hp * 128 : (hp + 1) * 128
                ],
            )
            return {"q": wqh, "k": wkh}

        def make_qk_units(hp, wtiles):
            """QK projection split into half-units (4 matmuls each) for
            fine-grained filler pacing."""
            qt = qtkt.tile([128, T], BF16, name="qtP", tag="qtP")
            kt = qtkt.tile([128, T], BF16, name="ktP", tag="ktP")
            units = []
            for t4 in range(4):
                for mat, dst in (("q", qt), ("k", kt)):
                    box = {}
                    wt = wtiles[mat]

                    def unit_a(wt=wt, t4=t4, box=box):
                        ps = pfl.tile([128, 512], F32, name="fill", tag="fill")
                        box["ps"] = ps
                        for c in range(4):
                            nc.tensor.matmul(
                                ps,
                                wt[:, c * 128 : (c + 1) * 128],
                                XTc(c, t4 * 512, (t4 + 1) * 512),
                                start=(c == 0),
                                stop=False,
                            )

                    def unit_b(wt=wt, dst=dst, t4=t4, box=box):
                        ps = box["ps"]
                        for c in range(4, 8):
                            nc.tensor.matmul(
                                ps,
                                wt[:, c * 128 : (c + 1) * 128],
                                XTc(c, t4 * 512, (t4 + 1) * 512),
                                start=False,
                                stop=(c == 7),
                            )
                        nc.vector.tensor_copy(
                            dst[:, t4 * 512 : (t4 + 1) * 512], ps
                        )

                    units.append(unit_a)
                    units.append(unit_b)
            return qt, kt, units

        # ---------- proj units (tail / fillers for pair 3) ----------
        def proj_units(tb):
            ot = {}
            def unit_ch(ch):
                def unit():
                    if ch == 0:
                        ot["t"] = osb.tile([128, C], BF16, name="ot", tag="ot")
                    ps = pfl.tile([128, 512], F32, name="fill", tag="fill")
                    for cb in range(4):
                        nc.tensor.matmul(
                            ps,
                            YT[cb][:, tb * 128 : (tb + 1) * 128],
                            WPall[:, cb * C + ch * 512 : cb * C + (ch + 1) * 512],
                            start=(cb == 0),
                            stop=(cb == 3),
                        )
                    nc.vector.tensor_copy(
                        ot["t"][:, ch * 512 : (ch + 1) * 512], ps
                    )
                    nc.sync.dma_start(
                        out=out[
                            tb * 128 : (tb + 1) * 128,
                            ch * 512 : (ch + 1) * 512,
                        ],
                        in_=ot["t"][:, ch * 512 : (ch + 1) * 512],
                    )
                return unit
            return [unit_ch(0), unit_ch(1)]

        def tail_units(qc):
            units = []
            for tb in range(qc * 4, qc * 4 + 4):
                units.extend(proj_units(tb))
            return units

        # ---------------- attention ----------------
        fill_q = deque()
        credit = [0.0]
        remkb = [40]

        def pump(n):
            for _ in range(min(n, len(fill_q))):
                fill_q.popleft()()

        def pump_paced():
            # adaptive: spread the current queue over the k-blocks left in
            # this head-pair so the PE never starves near the hp boundary
            if remkb[0] > 0:
                credit[0] += len(fill_q) / remkb[0]
                remkb[0] -= 1
            n = int(credit[0])
            if n > 0:
                n = min(n, len(fill_q))
                credit[0] -= n
                pump(n)

        def attention(hp, qt, kt, qc):
            q0 = qc * QCH
            nkb = (qc + 1) * 4
            hA, hB = 2 * hp, 2 * hp + 1
            ytA = psy.tile([128, QCH], F32, name="ytA", tag="ytA")
            ytB = psy.tile([128, QCH], F32, name="ytB", tag="ytB")

            def emit_av(idx, pt, off, w, kb):
                nc.tensor.matmul(
                    ytA[:, off : off + w],
                    VA[kb][:, hA * 128 : hA * 128 + 128],
                    pt[:, off : off + w],
                    start=(idx == 0),
                    stop=(idx == nkb - 1),
                )
                nc.tensor.matmul(
                    ytB[:, off : off + w],
                    VA[kb][:, hB * 128 : hB * 128 + 128],
                    pt[:, 512 + off : 512 + off + w],
                    start=(idx == 0),
                    stop=(idx == nkb - 1),
                )

            pend = deque()
            # diagonal (masked) blocks first: their gpsimd mask-muls finish
            # early, so the end-of-qc AV flush never waits on the mask
            kb_order = list(range(qc * 4, nkb)) + list(range(0, qc * 4))
            for idx, kb in enumerate(kb_order):
                j = kb - qc * 4
                off = j * 128 if j >= 1 else 0
                w = 512 - off
                ksl = slice(kb * KBLK, (kb + 1) * KBLK)
                sAB = pss.tile([128, 1024], F32, name="sAB", tag="sAB")
                nc.tensor.matmul(
                    sAB[:, off : 512],
                    kt[0:64, ksl],
                    qt[0:64, q0 + off : q0 + QCH],
                    start=True,
                    stop=True,
                    tile_position=(0, 0),
                )
                nc.tensor.matmul(
                    sAB[:, 512 + off : 1024],
                    kt[64:128, ksl],
                    qt[64:128, q0 + off : q0 + QCH],
                    start=True,
                    stop=True,
                    tile_position=(64, 0),
                )
                pt = ptp.tile([128, 1024], BF16, name="pt", tag="pt")
                nc.scalar.activation(
                    pt[:, off:1024], sAB[:, off:1024], EXP, scale=0.125
                )
                if j >= 0:
                    pv = pt.rearrange("p (s q) -> p s q", s=2)[
                        :, :, off : off + 128
                    ]
                    nc.gpsimd.tensor_mul(
                        pv, pv, MSK.rearrange("p (s q) -> p s q", s=2)
                    )
                pump_paced()
                if len(pend) == AVLAG:
                    emit_av(*pend.popleft())
                pend.append((idx, pt, off, w, kb))
            while pend:
                emit_av(*pend.popleft())
                # keep the PE fed while the flush AVs wait on the last exps
                if len(pend) % 2 == 1:
                    pump(1)
            lr2 = LR2[(hp * 4 + qc) % 2]
            yslA = YT[hp][0:64, q0 : q0 + QCH]
            yslB = YT[hp][64:128, q0 : q0 + QCH]
            for sub, (yt, ysl) in enumerate(((ytA, yslA), (ytB, yslB))):
                nc.vector.tensor_copy(ysl, yt[0:64, :])
                lf = nrm.tile([1, 512], F32, name="lf", tag="lf")
                nc.vector.tensor_copy(lf, yt[64:65, :])
                lf2 = nrm.tile([1, 512], F32, name="lf2", tag="lf2")
                nc.vector.reciprocal_approx_fast(lf2, lf)
                nc.vector.tensor_copy(lr2[sub * 64 : sub * 64 + 1, :], lf2)

            def norm_fin(yslA=yslA, yslB=yslB, lr2=lr2):
                rb = pfl.tile([128, 512], F32, name="fill", tag="fill")
                nc.tensor.matmul(rb, SEL, lr2, start=True, stop=True)
                nc.vector.tensor_mul(yslA, yslA, rb[0:64, :])
                nc.vector.tensor_mul(yslB, yslB, rb[64:128, :])

            fill_q.appendleft(norm_fin)

        # ---------------- main schedule ----------------
        wtiles0 = emit_w_slices(0, nc.sync, nc.sync)
        nc.sync.dma_start(
            out=WPall.rearrange("p (b c) -> p b c", c=C),
            in_=wp.rearrange("(b p) c -> p b c", p=128),
        )
        v_units = make_v_units()
        for u in v_units:
            u()
        qt, kt, units = make_qk_units(0, wtiles0)
        for u in units:
            u()
        for hp in range(4):
            nqt = nkt = None
            remkb[0] = 40
            credit[0] = 0.0
            if hp < 3:
                nwt = emit_w_slices(hp + 1, nc.sync, nc.sync)
                nqt, nkt, nunits = make_qk_units(hp + 1, nwt)
                fill_q.extend(nunits)
            for qc in (3, 2, 1, 0):
                attention(hp, qt, kt, qc)
                if hp == 3 and qc >= 1:
                    fill_q.extend(tail_units(qc))
            pump(len(fill_q))
            if hp < 3:
                qt, kt = nqt, nkt
        for u in tail_units(0):
            u()

    nc.compile()
    return nc


_NC_CACHE = None


def kernel(x0, w_attn, w_proj, _trace=False, _tmpdir=None):
    global _NC_CACHE
    import ml_dtypes

    from concourse.bass_utils import run_bass_kernel_spmd

    BF = ml_dtypes.bfloat16
    x0 = np.asarray(x0, dtype=np.float32)
    w_attn = np.asarray(w_attn, dtype=np.float32)
    w_proj = np.asarray(w_proj, dtype=np.float32)
    B = x0.shape[0]

    if _NC_CACHE is None:
        _NC_CACHE = _build_nc()
    nc = _NC_CACHE

    tri = np.triu(np.ones((128, 128), dtype=np.float32))
    msk = np.concatenate([tri, tri], axis=1).astype(BF)
    in_maps = []
    for core in range(8):
        b, g = divmod(core, 2)
        in_maps.append(
            {
                "xT": np.ascontiguousarray(x0[b].T).astype(BF),
                "wq": np.ascontiguousarray(
                    w_attn[:, g * G : (g + 1) * G]
                ).astype(BF),
                "wk": np.ascontiguousarray(
                    w_attn[:, C + g * G : C + (g + 1) * G]
                ).astype(BF),
                "wv": np.ascontiguousarray(
                    w_attn[:, 2 * C + g * G : 2 * C + (g + 1) * G]
                ).astype(BF),
                "wp": np.ascontiguousarray(
                    w_proj[g * G : (g + 1) * G, :]
                ).astype(BF),
                "mask": msk,
            }
        )

    res = run_bass_kernel_spmd(
        nc, in_maps, list(range(8)), trace=_trace, tmpdir=_tmpdir
    )
    outp = np.empty((B, T, C), dtype=np.float32)
    for b in range(B):
        outp[b] = res.results[2 * b]["out"].astype(np.float32) + res.results[
            2 * b + 1
        ]["out"].astype(np.float32)
    if _trace:
        kernel.last_exec_time_ns = res.exec_time_ns
    return outp


# revision 32
# speedup vs baseline: 1.0874x; 1.0142x over previous
"""Causal self-attention (B=4, T=2048, C=1024, H=16) on 8 trn2 NeuronCores.

Sharding: core = (batch b, head-group g), b in 0..3, g in 0..1. Each core does
8 heads of one batch element (Megatron column split of w_attn, row split of
w_proj); host sums the two partial projection outputs per batch element.

Per-core kernel, v4 (reduced PE work + coarse DMA):
 - All DRAM inputs bf16 (host casts); attention matmul operands bf16.
 - Coarse multi-c-block DMA descriptors issued in parallel from four engine
   queues at start (the v3 per-chunk DMAs serialized ~600ns/issue on sync and
   starved the V phase).
 - Q^T,K^T computed transposed (lhsT=W-block, rhs=x^T-block) so attention
   needs no transposes; V natural with a ones column per head so the
   attention AV matmul accumulates the softmax denominator l for free.
 - Attention per head-pair: S^T for both heads row-tiled into one
   [128,1024] PSUM tile per k-block; one exp (scale=1/8 folded in, no
   max-subtraction -- scores are N(0,1)); causal mask only on diagonal
   blocks via one doubled-mask bf16 multiply ON GPSIMD; AV deferred four
   k-blocks so exp latency hides; filler units (next head-pair's QK
   projection, output projection) credit-paced into every k-block.
 - qc processed descending (3,2,1,0): the long qc pipelines come first and
   the tiny qc0 flush lands where fillers still exist; hp3's projection
   tail units enqueue one qc earlier.
 - Normalization: l rows copied off PSUM + reciprocal on DVE, then ONE
   fp32r matmul per qc (contraction-2 selection matrix) builds the [128,512]
   broadcast of 1/l for BOTH heads (v3 used two rank-1 matmuls), and the Y
   drain is fused with the 1/l multiply in a single DVE tensor_tensor op
   per head (replaces v3's separate copy + in-place multiply).  The norm
   unit is pushed to the FRONT of the filler queue so the Y PSUM frees
   within a k-block or two.  (partition_broadcast was tried and reverted:
   it lives in a different gpsimd ucode library than tensor_tensor, and
   the resulting per-qc library swaps cost ~14us each.)
"""

import sys

if "/opt/trn_rl_repo" not in sys.path:
    sys.path.insert(0, "/opt/trn_rl_repo")

import numpy as np

T = 2048
C = 1024
G = 512          # per-core head-group width (8 heads x 64)
D = 64           # head dim
NH = 8           # heads per core
QCH = 512        # query chunk
KBLK = 128       # key block
AVLAG = 4        # AV deferral depth (k-blocks)


def _build_nc():
    from collections import deque
    from contextlib import ExitStack

    import concourse.bass as bass
    import concourse.mybir as mybir
    import concourse.tile as tile
    from concourse import bacc

    F32 = mybir.dt.float32
    F32R = mybir.dt.float32r
    BF16 = mybir.dt.bfloat16
    EXP = mybir.ActivationFunctionType.Exp

    nc = bacc.Bacc("TRN2", target_bir_lowering=False)

    xT = nc.dram_tensor("xT", [C, T], BF16, kind="ExternalInput")
    wq = nc.dram_tensor("wq", [C, G], BF16, kind="ExternalInput")
    wk = nc.dram_tensor("wk", [C, G], BF16, kind="ExternalInput")
    wv = nc.dram_tensor("wv", [C, G], BF16, kind="ExternalInput")
    wp = nc.dram_tensor("wp", [G, C], BF16, kind="ExternalInput")
    mask = nc.dram_tensor("mask", [128, 256], BF16, kind="ExternalInput")
    out = nc.dram_tensor("out", [T, C], BF16, kind="ExternalOutput")

    with tile.TileContext(nc) as tc, ExitStack() as ctx:
        persist = ctx.enter_context(tc.tile_pool(name="persist", bufs=1))
        xw = ctx.enter_context(tc.tile_pool(name="xw", bufs=1))
        wsl = ctx.enter_context(tc.tile_pool(name="wsl", bufs=2))
        qtkt = ctx.enter_context(tc.tile_pool(name="qtkt", bufs=2))
        ptp = ctx.enter_context(tc.tile_pool(name="ptp", bufs=7))
        nrm = ctx.enter_context(tc.tile_pool(name="nrm", bufs=2))
        osb = ctx.enter_context(tc.tile_pool(name="osb", bufs=2))
        wpp = ctx.enter_context(tc.tile_pool(name="wpp", bufs=1))
        pss = ctx.enter_context(tc.tile_pool(name="pss", bufs=2, space="PSUM"))
        psy = ctx.enter_context(tc.tile_pool(name="psy", bufs=1, space="PSUM"))
        pfl = ctx.enter_context(tc.tile_pool(name="pfl", bufs=2, space="PSUM"))

        VA = [persist.tile([128, NH * 128], BF16, name=f"va{i}", tag=f"va{i}")
              for i in range(16)]
        YT = [persist.tile([128, T], BF16, name=f"yt{i}", tag=f"yt{i}")
              for i in range(4)]
        MSK = persist.tile([128, 256], BF16, name="msk", tag="msk")
        ones_f32 = persist.tile([128, 64], F32, name="ones_f32", tag="ones_f32")
        # 65-deep contraction selection matrix: row 0 = e(0:64), row 64 =
        # e(64:128), rows 1..63 zero (single-partition writes must land on
        # partition bases 0/64, so the two 1/l rows live at 0 and 64)
        SEL = persist.tile([65, 128], BF16, name="sel", tag="sel")
        sel_f32 = persist.tile([65, 128], F32, name="sel_f32", tag="sel_f32")
        LR2 = [
            persist.tile([65, 512], BF16, name=f"lr2{i}", tag=f"lr2{i}")
            for i in range(2)
        ]

        # ---- coarse input DMA, issued across four engine queues ----
        XTall = xw.tile([128, 8 * T], BF16, name="xall", tag="xall")
        WVall = wsl.tile([128, 8 * G], BF16, name="wvall", tag="wvall")
        WPall = wpp.tile([128, 4 * C], BF16, name="wpall", tag="wpall")

        xt_s = XTall.rearrange("p (c t) -> p c t", t=T)
        xt_d = xT.rearrange("(c p) t -> p c t", p=128)
        wv_s = WVall.rearrange("p (c g) -> p c g", g=G)
        wv_d = wv.rearrange("(c p) g -> p c g", p=128)

        # ALL input DMAs ride the sync engine's single hardware queue in
        # strict need-order: one queue transfers in order at ~330 GB/s, so
        # in-order issue IS the bandwidth prioritization.  (Spreading across
        # engines was tried: concurrent queues share the same ~330 GB/s and
        # late-needed bulk blocks starved the critical first x columns.)
        nc.sync.dma_start(out=wv_s[:, 0:1, :], in_=wv_d[:, 0:1, :])
        nc.sync.dma_start(out=xt_s[:, 0:4, 0:128], in_=xt_d[:, 0:4, 0:128])
        nc.sync.dma_start(out=wv_s[:, 1:4, :], in_=wv_d[:, 1:4, :])
        nc.sync.dma_start(out=xt_s[:, 4:8, 0:128], in_=xt_d[:, 4:8, 0:128])
        nc.sync.dma_start(out=wv_s[:, 4:8, :], in_=wv_d[:, 4:8, :])
        nc.sync.dma_start(out=xt_s[:, :, 128:512], in_=xt_d[:, :, 128:512])
        nc.sync.dma_start(
            out=xt_s[:, :, 512:1024], in_=xt_d[:, :, 512:1024]
        )
        nc.sync.dma_start(
            out=xt_s[:, :, 1024:1536], in_=xt_d[:, :, 1024:1536]
        )
        nc.sync.dma_start(
            out=xt_s[:, :, 1536:2048], in_=xt_d[:, :, 1536:2048]
        )
        nc.sync.dma_start(out=MSK, in_=mask[:, :])

        def XTc(c, a, b):
            return XTall[:, c * T + a : c * T + b]

        nc.vector.memset(ones_f32, 1.0)
        # selection matrix for the combined two-head norm broadcast matmul
        nc.vector.memset(sel_f32, 0.0)
        nc.vector.tensor_copy(sel_f32[0:1, 0:64], ones_f32[0:1, 0:64])
        nc.vector.tensor_copy(sel_f32[64:65, 64:128], ones_f32[0:1, 0:64])
        nc.vector.tensor_copy(SEL, sel_f32)
        nc.vector.memset(LR2[0], 0.0)
        nc.vector.memset(LR2[1], 0.0)

        # V-augmentation ones columns
        ones_col = ones_f32[:, 0:8].rearrange("p (h o) -> p h o", o=1)
        for tb in range(16):
            vdst = VA[tb].rearrange("p (h e) -> p h e", e=128)[:, :, 64:65]
            nc.vector.tensor_copy(vdst, ones_col)

        # ---------------- phase 0: V ----------------
        v_done = [0]

        def make_v_units():
            units = []

            def unit_tb(tb):
                def unit():
                    ps = pfl.tile([128, 512], F32, name="fill", tag="fill")
                    for c in range(8):
                        nc.tensor.matmul(
                            ps,
                            XTc(c, tb * 128, (tb + 1) * 128),
                            WVall[:, c * G : (c + 1) * G],
                            start=(c == 0),
                            stop=(c == 7),
                        )
                    vdst = VA[tb].rearrange("p (h e) -> p h e", e=128)[
                        :, :, 0:64
                    ]
                    nc.vector.tensor_copy(
                        vdst, ps.rearrange("p (h d) -> p h d", d=64)
                    )
                    v_done[0] += 1
                return unit

            for tb in range(16):
                units.append(unit_tb(tb))
            return units

        # ---------------- QK machinery ----------------
        def emit_w_slices(hp, engine_q, engine_k):
            wqh = wsl.tile([128, 8 * 128], BF16, name="wqh", tag="wqh")
            wkh = wsl.tile([128, 8 * 128], BF16, name="wkh", tag="wkh")
            engine_q.dma_start(
                out=wqh.rearrange("p (c h) -> p c h", h=128),
                in_=wq.rearrange("(c p) g -> p c g", p=128)[
                    :, :, hp * 128 : (hp + 1) * 128
                ],
            )
            engine_k.dma_start(
                out=wkh.rearrange("p (c h) -> p c h", h=128),
                in_=wk.rearrange("(c p) g -> p c g", p=128)[
                    :, :, hp * 128 : (hp + 1) * 128
                ],
            )
            return {"q": wqh, "k": wkh}

        def make_qk_units(hp, wtiles):
            """QK projection split into half-units (4 matmuls each) for
            fine-grained filler pacing."""
            qt = qtkt.tile([128, T], BF16, name="qtP", tag="qtP")
            kt = qtkt.tile([128, T], BF16, name="ktP", tag="ktP")
            units = []
            for t4 in range(4):
                for mat, dst in (("q", qt), ("k", kt)):
                    box = {}
                    wt = wtiles[mat]

                    def unit_a(wt=wt, t4=t4, box=box):
                        ps = pfl.tile([128, 512], F32, name="fill", tag="fill")
                        box["ps"] = ps
                        for c in range(4):
                            nc.tensor.matmul(
                                ps,
                                wt[:, c * 128 : (c + 1) * 128],
                                XTc(c, t4 * 512, (t4 + 1) * 512),
                                start=(c == 0),
                                stop=False,
                            )

                    def unit_b(wt=wt, dst=dst, t4=t4, box=box):
                        ps = box["ps"]
                        for c in range(4, 8):
                            nc.tensor.matmul(
                                ps,
                                wt[:, c * 128 : (c + 1) * 128],
                                XTc(c, t4 * 512, (t4 + 1) * 512),
                                start=False,
                                stop=(c == 7),
                            )
                        nc.vector.tensor_copy(
                            dst[:, t4 * 512 : (t4 + 1) * 512], ps
                        )

                    units.append(unit_a)
                    units.append(unit_b)
            return qt, kt, units

        # ---------- proj units (tail / fillers for pair 3) ----------
        def proj_units(tb):
            ot = {}
            def unit_ch(ch):
                def unit():
                    if ch == 0:
                        ot["t"] = osb.tile([128, C], BF16, name="ot", tag="ot")
                    ps = pfl.tile([128, 512], F32, name="fill", tag="fill")
                    for cb in range(4):
                        nc.tensor.matmul(
                            ps,
                            YT[cb][:, tb * 128 : (tb + 1) * 128],
                            WPall[:, cb * C + ch * 512 : cb * C + (ch + 1) * 512],
                            start=(cb == 0),
                            stop=(cb == 3),
                        )
                    nc.vector.tensor_copy(
                        ot["t"][:, ch * 512 : (ch + 1) * 512], ps
                    )
                    nc.sync.dma_start(
                        out=out[
                            tb * 128 : (tb + 1) * 128,
                            ch * 512 : (ch + 1) * 512,
                        ],
                        in_=ot["t"][:, ch * 512 : (ch + 1) * 512],
                    )
                return unit
            return [unit_ch(0), unit_ch(1)]

        def tail_units(qc):
            units = []
            for tb in range(qc * 4, qc * 4 + 4):
                units.extend(proj_units(tb))
            return units

        # ---------------- attention ----------------
        fill_q = deque()
        credit = [0.0]
        remkb = [40]

        def pump(n):
            for _ in range(min(n, len(fill_q))):
                fill_q.popleft()()

        def pump_paced():
            # adaptive: spread the current queue over the k-blocks left in
            # this head-pair so the PE never starves near the hp boundary
            if remkb[0] > 0:
                credit[0] += len(fill_q) / remkb[0]
                remkb[0] -= 1
            n = int(credit[0])
            if n > 0:
                n = min(n, len(fill_q))
                credit[0] -= n
                pump(n)

        def attention(hp, qt, kt, qc):
            q0 = qc * QCH
            nkb = (qc + 1) * 4
            hA, hB = 2 * hp, 2 * hp + 1
            ytA = psy.tile([128, QCH], F32, name="ytA", tag="ytA")
            ytB = psy.tile([128, QCH], F32, name="ytB", tag="ytB")

            def emit_av(idx, pt, off, w, kb):
                nc.tensor.matmul(
                    ytA[:, off : off + w],
                    VA[kb][:, hA * 128 : hA * 128 + 128],
                    pt[:, off : off + w],
                    start=(idx == 0),
                    stop=(idx == nkb - 1),
                )
                nc.tensor.matmul(
                    ytB[:, off : off + w],
                    VA[kb][:, hB * 128 : hB * 128 + 128],
                    pt[:, 512 + off : 512 + off + w],
                    start=(idx == 0),
                    stop=(idx == nkb - 1),
                )

            pend = deque()
            # absorb the qc-start scalar backlog (previous qc's flush exps)
            pump(2)
            # diagonal (masked) blocks early -- their gpsimd mask-muls must
            # finish before their deferred AVs -- but SPACED among full
            # blocks: emitting all four trimmed diag S's back-to-back floods
            # the scalar queue with a burst of exps and the early AVs stall
            diag = list(range(qc * 4, nkb))
            full = list(range(0, qc * 4))
            kb_order = []
            for i in range(max(len(diag), len(full))):
                if i < len(diag):
                    kb_order.append(diag[i])
                if i < len(full):
                    kb_order.append(full[i])
            for idx, kb in enumerate(kb_order):
                j = kb - qc * 4
                off = j * 128 if j >= 1 else 0
                w = 512 - off
                ksl = slice(kb * KBLK, (kb + 1) * KBLK)
                sAB = pss.tile([128, 1024], F32, name="sAB", tag="sAB")
                nc.tensor.matmul(
                    sAB[:, off : 512],
                    kt[0:64, ksl],
                    qt[0:64, q0 + off : q0 + QCH],
                    start=True,
                    stop=True,
                    tile_position=(0, 0),
                )
                nc.tensor.matmul(
                    sAB[:, 512 + off : 1024],
                    kt[64:128, ksl],
                    qt[64:128, q0 + off : q0 + QCH],
                    start=True,
                    stop=True,
                    tile_position=(64, 0),
                )
                pt = ptp.tile([128, 1024], BF16, name="pt", tag="pt")
                nc.scalar.activation(
                    pt[:, off:1024], sAB[:, off:1024], EXP, scale=0.125
                )
                if j >= 0:
                    pv = pt.rearrange("p (s q) -> p s q", s=2)[
                        :, :, off : off + 128
                    ]
                    nc.gpsimd.tensor_mul(
                        pv, pv, MSK.rearrange("p (s q) -> p s q", s=2)
                    )
                pump_paced()
                if len(pend) == AVLAG:
                    emit_av(*pend.popleft())
                pend.append((idx, pt, off, w, kb))
            while pend:
                emit_av(*pend.popleft())
                # keep the PE fed while the flush AVs wait on the last exps
                if len(pend) % 2 == 1:
                    pump(1)
            lr2 = LR2[(hp * 4 + qc) % 2]
            yslA = YT[hp][0:64, q0 : q0 + QCH]
            yslB = YT[hp][64:128, q0 : q0 + QCH]
            for sub, (yt, ysl) in enumerate(((ytA, yslA), (ytB, yslB))):
                nc.vector.tensor_copy(ysl, yt[0:64, :])
                lf = nrm.tile([1, 512], F32, name="lf", tag="lf")
                nc.vector.tensor_copy(lf, yt[64:65, :])
                lf2 = nrm.tile([1, 512], F32, name="lf2", tag="lf2")
                nc.vector.reciprocal_approx_fast(lf2, lf)
                nc.vector.tensor_copy(lr2[sub * 64 : sub * 64 + 1, :], lf2)

            def norm_fin(yslA=yslA, yslB=yslB, lr2=lr2):
                rb = pfl.tile([128, 512], F32, name="fill", tag="fill")
                nc.tensor.matmul(rb, SEL, lr2, start=True, stop=True)
                nc.vector.tensor_mul(yslA, yslA, rb[0:64, :])
                nc.vector.tensor_mul(yslB, yslB, rb[64:128, :])

            fill_q.appendleft(norm_fin)

        # ---------------- main schedule ----------------
        wtiles0 = emit_w_slices(0, nc.sync, nc.sync)
        nc.sync.dma_start(
            out=WPall.rearrange("p (b c) -> p b c", c=C),
            in_=wp.rearrange("(b p) c -> p b c", p=128),
        )
        v_units = make_v_units()
        for u in v_units:
            u()
        qt, kt, units = make_qk_units(0, wtiles0)
        for u in units:
            u()
        for hp in range(4):
            nqt = nkt = None
            remkb[0] = 40
            credit[0] = 0.0
            if hp < 3:
                nwt = emit_w_slices(hp + 1, nc.sync, nc.sync)
                nqt, nkt, nunits = make_qk_units(hp + 1, nwt)
                fill_q.extend(nunits)
            for qc in (3, 2, 1, 0):
                attention(hp, qt, kt, qc)
                if hp == 3 and qc >= 1:
                    fill_q.extend(tail_units(qc))
            pump(len(fill_q))
            if hp < 3:
                qt, kt = nqt, nkt
        for u in tail_units(0):
            u()

    nc.compile()
    return nc


_NC_CACHE = None


def kernel(x0, w_attn, w_proj, _trace=False, _tmpdir=None):
    global _NC_CACHE
    import ml_dtypes

    from concourse.bass_utils import run_bass_kernel_spmd

    BF = ml_dtypes.bfloat16
    x0 = np.asarray(x0, dtype=np.float32)
    w_attn = np.asarray(w_attn, dtype=np.float32)
    w_proj = np.asarray(w_proj, dtype=np.float32)
    B = x0.shape[0]

    if _NC_CACHE is None:
        _NC_CACHE = _build_nc()
    nc = _NC_CACHE

    tri = np.triu(np.ones((128, 128), dtype=np.float32))
    msk = np.concatenate([tri, tri], axis=1).astype(BF)
    in_maps = []
    for core in range(8):
        b, g = divmod(core, 2)
        in_maps.append(
            {
                "xT": np.ascontiguousarray(x0[b].T).astype(BF),
                "wq": np.ascontiguousarray(
                    w_attn[:, g * G : (g + 1) * G]
                ).astype(BF),
                "wk": np.ascontiguousarray(
                    w_attn[:, C + g * G : C + (g + 1) * G]
                ).astype(BF),
                "wv": np.ascontiguousarray(
                    w_attn[:, 2 * C + g * G : 2 * C + (g + 1) * G]
                ).astype(BF),
                "wp": np.ascontiguousarray(
                    w_proj[g * G : (g + 1) * G, :]
                ).astype(BF),
                "mask": msk,
            }
        )

    res = run_bass_kernel_spmd(
        nc, in_maps, list(range(8)), trace=_trace, tmpdir=_tmpdir
    )
    outp = np.empty((B, T, C), dtype=np.float32)
    for b in range(B):
        outp[b] = res.results[2 * b]["out"].astype(np.float32) + res.results[
            2 * b + 1
        ]["out"].astype(np.float32)
    if _trace:
        kernel.last_exec_time_ns = res.exec_time_ns
    return outp


# revision 41
# speedup vs baseline: 1.1056x; 1.0168x over previous
"""Causal self-attention (B=4, T=2048, C=1024, H=16) on 8 trn2 NeuronCores.

Sharding: core = (batch b, head-group g), b in 0..3, g in 0..1. Each core does
8 heads of one batch element (Megatron column split of w_attn, row split of
w_proj); host sums the two partial projection outputs per batch element.

Per-core kernel, v4 (reduced PE work + coarse DMA):
 - All DRAM inputs bf16 (host casts); attention matmul operands bf16.
 - Coarse multi-c-block DMA descriptors issued in parallel from four engine
   queues at start (the v3 per-chunk DMAs serialized ~600ns/issue on sync and
   starved the V phase).
 - Q^T,K^T computed transposed (lhsT=W-block, rhs=x^T-block) so attention
   needs no transposes; V natural with a ones column per head so the
   attention AV matmul accumulates the softmax denominator l for free.
 - Attention per head-pair: S^T for both heads row-tiled into one
   [128,1024] PSUM tile per k-block; one exp (scale=1/8 folded in, no
   max-subtraction -- scores are N(0,1)); causal mask only on diagonal
   blocks via one doubled-mask bf16 multiply ON GPSIMD; AV deferred four
   k-blocks so exp latency hides; filler units (next head-pair's QK
   projection, output projection) credit-paced into every k-block.
 - qc processed descending (3,2,1,0): the long qc pipelines come first and
   the tiny qc0 flush lands where fillers still exist; hp3's projection
   tail units enqueue one qc earlier.
 - Normalization: l rows copied off PSUM + reciprocal on DVE, then ONE
   fp32r matmul per qc (contraction-2 selection matrix) builds the [128,512]
   broadcast of 1/l for BOTH heads (v3 used two rank-1 matmuls), and the Y
   drain is fused with the 1/l multiply in a single DVE tensor_tensor op
   per head (replaces v3's separate copy + in-place multiply).  The norm
   unit is pushed to the FRONT of the filler queue so the Y PSUM frees
   within a k-block or two.  (partition_broadcast was tried and reverted:
   it lives in a different gpsimd ucode library than tensor_tensor, and
   the resulting per-qc library swaps cost ~14us each.)
"""

import sys

if "/opt/trn_rl_repo" not in sys.path:
    sys.path.insert(0, "/opt/trn_rl_repo")

import numpy as np

T = 2048
C = 1024
G = 512          # per-core head-group width (8 heads x 64)
D = 64           # head dim
NH = 8           # heads per core
QCH = 512        # query chunk
KBLK = 128       # key block
AVLAG = 5        # AV deferral depth (k-blocks)


def _build_nc():
    from collections import deque
    from contextlib import ExitStack

    import concourse.bass as bass
    import concourse.mybir as mybir
    import concourse.tile as tile
    from concourse import bacc

    F32 = mybir.dt.float32
    F32R = mybir.dt.float32r
    BF16 = mybir.dt.bfloat16
    EXP = mybir.ActivationFunctionType.Exp

    nc = bacc.Bacc("TRN2", target_bir_lowering=False)

    xT = nc.dram_tensor("xT", [C, T], BF16, kind="ExternalInput")
    wq = nc.dram_tensor("wq", [C, G], BF16, kind="ExternalInput")
    wk = nc.dram_tensor("wk", [C, G], BF16, kind="ExternalInput")
    wv = nc.dram_tensor("wv", [C, G], BF16, kind="ExternalInput")
    wp = nc.dram_tensor("wp", [G, C], BF16, kind="ExternalInput")
    mask = nc.dram_tensor("mask", [128, 256], BF16, kind="ExternalInput")
    out = nc.dram_tensor("out", [T, C], BF16, kind="ExternalOutput")

    with tile.TileContext(nc) as tc, ExitStack() as ctx:
        persist = ctx.enter_context(tc.tile_pool(name="persist", bufs=1))
        xw = ctx.enter_context(tc.tile_pool(name="xw", bufs=1))
        wsl = ctx.enter_context(tc.tile_pool(name="wsl", bufs=2))
        qtkt = ctx.enter_context(tc.tile_pool(name="qtkt", bufs=2))
        ptp = ctx.enter_context(tc.tile_pool(name="ptp", bufs=9))
        nrm = ctx.enter_context(tc.tile_pool(name="nrm", bufs=2))
        osb = ctx.enter_context(tc.tile_pool(name="osb", bufs=2))
        wpp = ctx.enter_context(tc.tile_pool(name="wpp", bufs=1))
        pss = ctx.enter_context(tc.tile_pool(name="pss", bufs=2, space="PSUM"))
        psy = ctx.enter_context(tc.tile_pool(name="psy", bufs=1, space="PSUM"))
        pfl = ctx.enter_context(tc.tile_pool(name="pfl", bufs=2, space="PSUM"))

        VA = [persist.tile([128, NH * 128], BF16, name=f"va{i}", tag=f"va{i}")
              for i in range(16)]
        YT = [persist.tile([128, T], BF16, name=f"yt{i}", tag=f"yt{i}")
              for i in range(4)]
        MSK = persist.tile([128, 256], BF16, name="msk", tag="msk")
        ones_f32 = persist.tile([128, 64], F32, name="ones_f32", tag="ones_f32")
        # 65-deep contraction selection matrix: row 0 = e(0:64), row 64 =
        # e(64:128), rows 1..63 zero (single-partition writes must land on
        # partition bases 0/64, so the two 1/l rows live at 0 and 64)
        SEL = persist.tile([65, 128], BF16, name="sel", tag="sel")
        sel_f32 = persist.tile([65, 128], F32, name="sel_f32", tag="sel_f32")
        LR2 = [
            persist.tile([65, 512], BF16, name=f"lr2{i}", tag=f"lr2{i}")
            for i in range(2)
        ]

        # ---- coarse input DMA, issued across four engine queues ----
        XTall = xw.tile([128, 8 * T], BF16, name="xall", tag="xall")
        WVall = wsl.tile([128, 8 * G], BF16, name="wvall", tag="wvall")
        WPall = wpp.tile([128, 4 * C], BF16, name="wpall", tag="wpall")

        xt_s = XTall.rearrange("p (c t) -> p c t", t=T)
        xt_d = xT.rearrange("(c p) t -> p c t", p=128)
        wv_s = WVall.rearrange("p (c g) -> p c g", g=G)
        wv_d = wv.rearrange("(c p) g -> p c g", p=128)

        # ALL input DMAs ride the sync engine's single hardware queue in
        # strict need-order: one queue transfers in order at ~330 GB/s, so
        # in-order issue IS the bandwidth prioritization.  (Spreading across
        # engines was tried: concurrent queues share the same ~330 GB/s and
        # late-needed bulk blocks starved the critical first x columns.)
        nc.sync.dma_start(out=wv_s[:, 0:1, :], in_=wv_d[:, 0:1, :])
        nc.sync.dma_start(out=xt_s[:, 0:4, 0:128], in_=xt_d[:, 0:4, 0:128])
        nc.sync.dma_start(out=wv_s[:, 1:4, :], in_=wv_d[:, 1:4, :])
        nc.sync.dma_start(out=xt_s[:, 4:8, 0:128], in_=xt_d[:, 4:8, 0:128])
        nc.sync.dma_start(out=wv_s[:, 4:8, :], in_=wv_d[:, 4:8, :])
        nc.sync.dma_start(out=xt_s[:, :, 128:512], in_=xt_d[:, :, 128:512])
        nc.sync.dma_start(
            out=xt_s[:, :, 512:1024], in_=xt_d[:, :, 512:1024]
        )
        nc.sync.dma_start(
            out=xt_s[:, :, 1024:1536], in_=xt_d[:, :, 1024:1536]
        )
        nc.sync.dma_start(
            out=xt_s[:, :, 1536:2048], in_=xt_d[:, :, 1536:2048]
        )
        nc.sync.dma_start(out=MSK, in_=mask[:, :])

        def XTc(c, a, b):
            return XTall[:, c * T + a : c * T + b]

        nc.vector.memset(ones_f32, 1.0)
        # selection matrix for the combined two-head norm broadcast matmul
        nc.vector.memset(sel_f32, 0.0)
        nc.vector.tensor_copy(sel_f32[0:1, 0:64], ones_f32[0:1, 0:64])
        nc.vector.tensor_copy(sel_f32[64:65, 64:128], ones_f32[0:1, 0:64])
        nc.vector.tensor_copy(SEL, sel_f32)
        nc.vector.memset(LR2[0], 0.0)
        nc.vector.memset(LR2[1], 0.0)

        # V-augmentation ones columns
        ones_col = ones_f32[:, 0:8].rearrange("p (h o) -> p h o", o=1)
        for tb in range(16):
            vdst = VA[tb].rearrange("p (h e) -> p h e", e=128)[:, :, 64:65]
            nc.vector.tensor_copy(vdst, ones_col)

        # ---------------- phase 0: V ----------------
        v_done = [0]

        def make_v_units():
            units = []

            def unit_tb(tb):
                def unit():
                    ps = pfl.tile([128, 512], F32, name="fill", tag="fill")
                    for c in range(8):
                        nc.tensor.matmul(
                            ps,
                            XTc(c, tb * 128, (tb + 1) * 128),
                            WVall[:, c * G : (c + 1) * G],
                            start=(c == 0),
                            stop=(c == 7),
                        )
                    vdst = VA[tb].rearrange("p (h e) -> p h e", e=128)[
                        :, :, 0:64
                    ]
                    nc.vector.tensor_copy(
                        vdst, ps.rearrange("p (h d) -> p h d", d=64)
                    )
                    v_done[0] += 1
                return unit

            for tb in range(16):
                units.append(unit_tb(tb))
            return units

        # ---------------- QK machinery ----------------
        def emit_w_slices(hp, engine_q, engine_k):
            wqh = wsl.tile([128, 8 * 128], BF16, name="wqh", tag="wqh")
            wkh = wsl.tile([128, 8 * 128], BF16, name="wkh", tag="wkh")
            engine_q.dma_start(
                out=wqh.rearrange("p (c h) -> p c h", h=128),
                in_=wq.rearrange("(c p) g -> p c g", p=128)[
                    :, :, hp * 128 : (hp + 1) * 128
                ],
            )
            engine_k.dma_start(
                out=wkh.rearrange("p (c h) -> p c h", h=128),
                in_=wk.rearrange("(c p) g -> p c g", p=128)[
                    :, :, hp * 128 : (hp + 1) * 128
                ],
            )
            return {"q": wqh, "k": wkh}

        def make_qk_units(hp, wtiles):
            """QK projection split into half-units (4 matmuls each) for
            fine-grained filler pacing."""
            qt = qtkt.tile([128, T], BF16, name="qtP", tag="qtP")
            kt = qtkt.tile([128, T], BF16, name="ktP", tag="ktP")
            units = []
            for t4 in range(4):
                for mat, dst in (("q", qt), ("k", kt)):
                    box = {}
                    wt = wtiles[mat]

                    def unit_a(wt=wt, t4=t4, box=box):
                        ps = pfl.tile([128, 512], F32, name="fill", tag="fill")
                        box["ps"] = ps
                        for c in range(4):
                            nc.tensor.matmul(
                                ps,
                                wt[:, c * 128 : (c + 1) * 128],
                                XTc(c, t4 * 512, (t4 + 1) * 512),
                                start=(c == 0),
                                stop=False,
                            )

                    def unit_b(wt=wt, dst=dst, t4=t4, box=box):
                        ps = box["ps"]
                        for c in range(4, 8):
                            nc.tensor.matmul(
                                ps,
                                wt[:, c * 128 : (c + 1) * 128],
                                XTc(c, t4 * 512, (t4 + 1) * 512),
                                start=False,
                                stop=(c == 7),
                            )
                        nc.vector.tensor_copy(
                            dst[:, t4 * 512 : (t4 + 1) * 512], ps
                        )

                    units.append(unit_a)
                    units.append(unit_b)
            return qt, kt, units

        # ---------- proj units (tail / fillers for pair 3) ----------
        def proj_units(tb):
            ot = {}
            def unit_ch(ch):
                def unit():
                    if ch == 0:
                        ot["t"] = osb.tile([128, C], BF16, name="ot", tag="ot")
                    ps = pfl.tile([128, 512], F32, name="fill", tag="fill")
                    for cb in range(4):
                        nc.tensor.matmul(
                            ps,
                            YT[cb][:, tb * 128 : (tb + 1) * 128],
                            WPall[:, cb * C + ch * 512 : cb * C + (ch + 1) * 512],
                            start=(cb == 0),
                            stop=(cb == 3),
                        )
                    nc.vector.tensor_copy(
                        ot["t"][:, ch * 512 : (ch + 1) * 512], ps
                    )
                    nc.sync.dma_start(
                        out=out[
                            tb * 128 : (tb + 1) * 128,
                            ch * 512 : (ch + 1) * 512,
                        ],
                        in_=ot["t"][:, ch * 512 : (ch + 1) * 512],
                    )
                return unit
            return [unit_ch(0), unit_ch(1)]

        def tail_units(qc):
            units = []
            for tb in range(qc * 4, qc * 4 + 4):
                units.extend(proj_units(tb))
            return units

        # ---------------- attention ----------------
        fill_q = deque()
        credit = [0.0]
        remkb = [40]

        def pump(n):
            for _ in range(min(n, len(fill_q))):
                fill_q.popleft()()

        def pump_paced():
            # adaptive: spread the current queue over the k-blocks left in
            # this head-pair so the PE never starves near the hp boundary
            if remkb[0] > 0:
                credit[0] += len(fill_q) / remkb[0]
                remkb[0] -= 1
            n = int(credit[0])
            if n > 0:
                n = min(n, len(fill_q))
                credit[0] -= n
                pump(n)

        def flush_carry(carry, n=None):
            """Emit up to n (or all) deferred AVs from the previous qc; when
            the queue empties, emit that qc's drain + norm."""
            if carry is None:
                return None
            cpend, cemit, cfin = carry
            k = len(cpend) if n is None else min(n, len(cpend))
            for _ in range(k):
                cemit(*cpend.popleft())
                if n is None and len(cpend) % 2 == 1:
                    pump(1)
            if not cpend:
                cfin()
                return None
            return carry

        def attention(hp, qt, kt, qc, carry):
            q0 = qc * QCH
            nkb = (qc + 1) * 4
            hA, hB = 2 * hp, 2 * hp + 1
            # psy tiles allocated LAZILY at the first AV emission: eager
            # allocation here would predate the carried-in previous qc's AV
            # writes + drain reads of the same (bufs=1) buffers, and the
            # pool's WAR tracking would miss them -> race
            ytbox = {}

            def get_yts():
                if "A" not in ytbox:
                    ytbox["A"] = psy.tile(
                        [128, QCH], F32, name="ytA", tag="ytA"
                    )
                    ytbox["B"] = psy.tile(
                        [128, QCH], F32, name="ytB", tag="ytB"
                    )
                return ytbox["A"], ytbox["B"]

            def emit_av(idx, pt, off, w, kb):
                ytA, ytB = get_yts()
                nc.tensor.matmul(
                    ytA[:, off : off + w],
                    VA[kb][:, hA * 128 : hA * 128 + 128],
                    pt[:, off : off + w],
                    start=(idx == 0),
                    stop=(idx == nkb - 1),
                )
                nc.tensor.matmul(
                    ytB[:, off : off + w],
                    VA[kb][:, hB * 128 : hB * 128 + 128],
                    pt[:, 512 + off : 512 + off + w],
                    start=(idx == 0),
                    stop=(idx == nkb - 1),
                )

            pend = deque()
            # absorb the qc-start scalar backlog (previous qc's flush exps)
            pump(2)
            # diagonal (masked) blocks early -- their gpsimd mask-muls must
            # finish before their deferred AVs -- but SPACED among full
            # blocks: emitting all four trimmed diag S's back-to-back floods
            # the scalar queue with a burst of exps and the early AVs stall
            diag = list(range(qc * 4, nkb))
            full = list(range(0, qc * 4))
            kb_order = []
            for i in range(max(len(diag), len(full))):
                if i < len(diag):
                    kb_order.append(diag[i])
                if i < len(full):
                    kb_order.append(full[i])
            for idx, kb in enumerate(kb_order):
                j = kb - qc * 4
                off = j * 128 if j >= 1 else 0
                w = 512 - off
                ksl = slice(kb * KBLK, (kb + 1) * KBLK)
                sAB = pss.tile([128, 1024], F32, name="sAB", tag="sAB")
                nc.tensor.matmul(
                    sAB[:, off : 512],
                    kt[0:64, ksl],
                    qt[0:64, q0 + off : q0 + QCH],
                    start=True,
                    stop=True,
                    tile_position=(0, 0),
                )
                nc.tensor.matmul(
                    sAB[:, 512 + off : 1024],
                    kt[64:128, ksl],
                    qt[64:128, q0 + off : q0 + QCH],
                    start=True,
                    stop=True,
                    tile_position=(64, 0),
                )
                pt = ptp.tile([128, 1024], BF16, name="pt", tag="pt")
                nc.scalar.activation(
                    pt[:, off:1024], sAB[:, off:1024], EXP, scale=0.125
                )
                if j >= 0:
                    pv = pt.rearrange("p (s q) -> p s q", s=2)[
                        :, :, off : off + 128
                    ]
                    nc.gpsimd.tensor_mul(
                        pv, pv, MSK.rearrange("p (s q) -> p s q", s=2)
                    )
                pump_paced()
                carry = flush_carry(carry, 2)
                if len(pend) == AVLAG:
                    emit_av(*pend.popleft())
                pend.append((idx, pt, off, w, kb))

            def finish(hp=hp, qc=qc, q0=q0):
                ytA, ytB = get_yts()
                lr2 = LR2[(hp * 4 + qc) % 2]
                yslA = YT[hp][0:64, q0 : q0 + QCH]
                yslB = YT[hp][64:128, q0 : q0 + QCH]
                for sub, (yt, ysl) in enumerate(((ytA, yslA), (ytB, yslB))):
                    nc.vector.tensor_copy(ysl, yt[0:64, :])
                    lf = nrm.tile([1, 512], F32, name="lf", tag="lf")
                    nc.vector.tensor_copy(lf, yt[64:65, :])
                    lf2 = nrm.tile([1, 512], F32, name="lf2", tag="lf2")
                    nc.vector.reciprocal_approx_fast(lf2, lf)
                    nc.vector.tensor_copy(
                        lr2[sub * 64 : sub * 64 + 1, :], lf2
                    )

                def norm_fin(yslA=yslA, yslB=yslB, lr2=lr2, hp=hp, qc=qc):
                    rb = pfl.tile([128, 512], F32, name="fill", tag="fill")
                    nc.tensor.matmul(rb, SEL, lr2, start=True, stop=True)
                    nc.vector.tensor_mul(yslA, yslA, rb[0:64, :])
                    nc.vector.tensor_mul(yslB, yslB, rb[64:128, :])
                    if hp == 3:
                        # emitting the proj tail here (not earlier) makes
                        # "proj reads YT only after it is drained+normalized"
                        # hold by construction
                        fill_q.extend(tail_units(qc))

                fill_q.appendleft(norm_fin)

            return (pend, emit_av, finish)

        # ---------------- main schedule ----------------
        wtiles0 = emit_w_slices(0, nc.sync, nc.sync)
        nc.sync.dma_start(
            out=WPall.rearrange("p (b c) -> p b c", c=C),
            in_=wp.rearrange("(b p) c -> p b c", p=128),
        )
        v_units = make_v_units()
        for u in v_units:
            u()
        qt, kt, units = make_qk_units(0, wtiles0)
        for u in units:
            u()
        carry = None
        for hp in range(4):
            nqt = nkt = None
            remkb[0] = 40
            credit[0] = 0.0
            if hp < 3:
                nwt = emit_w_slices(hp + 1, nc.sync, nc.sync)
                nqt, nkt, nunits = make_qk_units(hp + 1, nwt)
                fill_q.extend(nunits)
            for qc in (3, 2, 1, 0):
                carry = attention(hp, qt, kt, qc, carry)
            if hp < 3:
                pump(len(fill_q))
                qt, kt = nqt, nkt
        carry = flush_carry(carry)
        while fill_q:
            fill_q.popleft()()

    nc.compile()
    return nc


_NC_CACHE = None


def kernel(x0, w_attn, w_proj, _trace=False, _tmpdir=None):
    global _NC_CACHE
    import ml_dtypes

    from concourse.bass_utils import run_bass_kernel_spmd

    BF = ml_dtypes.bfloat16
    x0 = np.asarray(x0, dtype=np.float32)
    w_attn = np.asarray(w_attn, dtype=np.float32)
    w_proj = np.asarray(w_proj, dtype=np.float32)
    B = x0.shape[0]

    if _NC_CACHE is None:
        _NC_CACHE = _build_nc()
    nc = _NC_CACHE

    tri = np.triu(np.ones((128, 128), dtype=np.float32))
    msk = np.concatenate([tri, tri], axis=1).astype(BF)
    in_maps = []
    for core in range(8):
        b, g = divmod(core, 2)
        in_maps.append(
            {
                "xT": np.ascontiguousarray(x0[b].T).astype(BF),
                "wq": np.ascontiguousarray(
                    w_attn[:, g * G : (g + 1) * G]
                ).astype(BF),
                "wk": np.ascontiguousarray(
                    w_attn[:, C + g * G : C + (g + 1) * G]
                ).astype(BF),
                "wv": np.ascontiguousarray(
                    w_attn[:, 2 * C + g * G : 2 * C + (g + 1) * G]
                ).astype(BF),
                "wp": np.ascontiguousarray(
                    w_proj[g * G : (g + 1) * G, :]
                ).astype(BF),
                "mask": msk,
            }
        )

    res = run_bass_kernel_spmd(
        nc, in_maps, list(range(8)), trace=_trace, tmpdir=_tmpdir
    )
    outp = np.empty((B, T, C), dtype=np.float32)
    for b in range(B):
        outp[b] = res.results[2 * b]["out"].astype(np.float32) + res.results[
            2 * b + 1
        ]["out"].astype(np.float32)
    if _trace:
        kernel.last_exec_time_ns = res.exec_time_ns
    return outp


# revision 45
# speedup vs baseline: 1.1191x; 1.0122x over previous
"""Causal self-attention (B=4, T=2048, C=1024, H=16) on 8 trn2 NeuronCores.

Sharding: core = (batch b, head-group g), b in 0..3, g in 0..1. Each core does
8 heads of one batch element (Megatron column split of w_attn, row split of
w_proj); host sums the two partial projection outputs per batch element.

Per-core kernel, v4 (reduced PE work + coarse DMA):
 - All DRAM inputs bf16 (host casts); attention matmul operands bf16.
 - Coarse multi-c-block DMA descriptors issued in parallel from four engine
   queues at start (the v3 per-chunk DMAs serialized ~600ns/issue on sync and
   starved the V phase).
 - Q^T,K^T computed transposed (lhsT=W-block, rhs=x^T-block) so attention
   needs no transposes; V natural with a ones column per head so the
   attention AV matmul accumulates the softmax denominator l for free.
 - Attention per head-pair: S^T for both heads row-tiled into one
   [128,1024] PSUM tile per k-block; one exp (scale=1/8 folded in, no
   max-subtraction -- scores are N(0,1)); causal mask only on diagonal
   blocks via one doubled-mask bf16 multiply ON GPSIMD; AV deferred four
   k-blocks so exp latency hides; filler units (next head-pair's QK
   projection, output projection) credit-paced into every k-block.
 - qc processed descending (3,2,1,0): the long qc pipelines come first and
   the tiny qc0 flush lands where fillers still exist; hp3's projection
   tail units enqueue one qc earlier.
 - Normalization: l rows copied off PSUM + reciprocal on DVE, then ONE
   fp32r matmul per qc (contraction-2 selection matrix) builds the [128,512]
   broadcast of 1/l for BOTH heads (v3 used two rank-1 matmuls), and the Y
   drain is fused with the 1/l multiply in a single DVE tensor_tensor op
   per head (replaces v3's separate copy + in-place multiply).  The norm
   unit is pushed to the FRONT of the filler queue so the Y PSUM frees
   within a k-block or two.  (partition_broadcast was tried and reverted:
   it lives in a different gpsimd ucode library than tensor_tensor, and
   the resulting per-qc library swaps cost ~14us each.)
"""

import sys

if "/opt/trn_rl_repo" not in sys.path:
    sys.path.insert(0, "/opt/trn_rl_repo")

import numpy as np

T = 2048
C = 1024
G = 512          # per-core head-group width (8 heads x 64)
D = 64           # head dim
NH = 8           # heads per core
QCH = 512        # query chunk
KBLK = 128       # key block
AVLAG = 5        # AV deferral depth (k-blocks)


def _build_nc():
    from collections import deque
    from contextlib import ExitStack

    import concourse.bass as bass
    import concourse.mybir as mybir
    import concourse.tile as tile
    from concourse import bacc

    F32 = mybir.dt.float32
    F32R = mybir.dt.float32r
    BF16 = mybir.dt.bfloat16
    EXP = mybir.ActivationFunctionType.Exp

    nc = bacc.Bacc("TRN2", target_bir_lowering=False)

    xT = nc.dram_tensor("xT", [C, T], BF16, kind="ExternalInput")
    wq = nc.dram_tensor("wq", [C, G], BF16, kind="ExternalInput")
    wk = nc.dram_tensor("wk", [C, G], BF16, kind="ExternalInput")
    wv = nc.dram_tensor("wv", [C, G], BF16, kind="ExternalInput")
    wp = nc.dram_tensor("wp", [G, C], BF16, kind="ExternalInput")
    mask = nc.dram_tensor("mask", [128, 256], BF16, kind="ExternalInput")
    out = nc.dram_tensor("out", [T, C], BF16, kind="ExternalOutput")

    with tile.TileContext(nc) as tc, ExitStack() as ctx:
        persist = ctx.enter_context(tc.tile_pool(name="persist", bufs=1))
        xw = ctx.enter_context(tc.tile_pool(name="xw", bufs=1))
        wsl = ctx.enter_context(tc.tile_pool(name="wsl", bufs=2))
        qtkt = ctx.enter_context(tc.tile_pool(name="qtkt", bufs=2))
        ptp = ctx.enter_context(tc.tile_pool(name="ptp", bufs=9))
        nrm = ctx.enter_context(tc.tile_pool(name="nrm", bufs=2))
        osb = ctx.enter_context(tc.tile_pool(name="osb", bufs=2))
        wpp = ctx.enter_context(tc.tile_pool(name="wpp", bufs=1))
        pss = ctx.enter_context(tc.tile_pool(name="pss", bufs=2, space="PSUM"))
        psy = ctx.enter_context(tc.tile_pool(name="psy", bufs=1, space="PSUM"))
        pfl = ctx.enter_context(tc.tile_pool(name="pfl", bufs=2, space="PSUM"))

        VA = [persist.tile([128, NH * 128], BF16, name=f"va{i}", tag=f"va{i}")
              for i in range(16)]
        YT = [persist.tile([128, T], BF16, name=f"yt{i}", tag=f"yt{i}")
              for i in range(4)]
        MSK = persist.tile([128, 256], BF16, name="msk", tag="msk")
        ones_f32 = persist.tile([128, 64], F32, name="ones_f32", tag="ones_f32")
        # 65-deep contraction selection matrix: row 0 = e(0:64), row 64 =
        # e(64:128), rows 1..63 zero (single-partition writes must land on
        # partition bases 0/64, so the two 1/l rows live at 0 and 64)
        SEL = persist.tile([65, 128], BF16, name="sel", tag="sel")
        sel_f32 = persist.tile([65, 128], F32, name="sel_f32", tag="sel_f32")
        LR2 = [
            persist.tile([65, 512], BF16, name=f"lr2{i}", tag=f"lr2{i}")
            for i in range(2)
        ]

        # ---- coarse input DMA, issued across four engine queues ----
        XTall = xw.tile([128, 8 * T], BF16, name="xall", tag="xall")
        WVall = wsl.tile([128, 8 * G], BF16, name="wvall", tag="wvall")
        WPall = wpp.tile([128, 4 * C], BF16, name="wpall", tag="wpall")

        xt_s = XTall.rearrange("p (c t) -> p c t", t=T)
        xt_d = xT.rearrange("(c p) t -> p c t", p=128)
        wv_s = WVall.rearrange("p (c g) -> p c g", g=G)
        wv_d = wv.rearrange("(c p) g -> p c g", p=128)

        # ALL input DMAs ride the sync engine's single hardware queue in
        # strict need-order: one queue transfers in order at ~330 GB/s, so
        # in-order issue IS the bandwidth prioritization.  (Spreading across
        # engines was tried: concurrent queues share the same ~330 GB/s and
        # late-needed bulk blocks starved the critical first x columns.)
        nc.sync.dma_start(out=wv_s[:, 0:1, :], in_=wv_d[:, 0:1, :])
        nc.sync.dma_start(out=xt_s[:, 0:4, 0:128], in_=xt_d[:, 0:4, 0:128])
        nc.sync.dma_start(out=wv_s[:, 1:4, :], in_=wv_d[:, 1:4, :])
        nc.sync.dma_start(out=xt_s[:, 4:8, 0:128], in_=xt_d[:, 4:8, 0:128])
        nc.sync.dma_start(out=wv_s[:, 4:8, :], in_=wv_d[:, 4:8, :])
        nc.sync.dma_start(out=xt_s[:, :, 128:256], in_=xt_d[:, :, 128:256])
        nc.sync.dma_start(out=xt_s[:, :, 256:512], in_=xt_d[:, :, 256:512])
        nc.sync.dma_start(
            out=xt_s[:, :, 512:1024], in_=xt_d[:, :, 512:1024]
        )
        nc.sync.dma_start(
            out=xt_s[:, :, 1024:1536], in_=xt_d[:, :, 1024:1536]
        )
        nc.sync.dma_start(
            out=xt_s[:, :, 1536:2048], in_=xt_d[:, :, 1536:2048]
        )
        nc.sync.dma_start(out=MSK, in_=mask[:, :])

        def XTc(c, a, b):
            return XTall[:, c * T + a : c * T + b]

        nc.vector.memset(ones_f32, 1.0)
        # selection matrix for the combined two-head norm broadcast matmul
        nc.vector.memset(sel_f32, 0.0)
        nc.vector.tensor_copy(sel_f32[0:1, 0:64], ones_f32[0:1, 0:64])
        nc.vector.tensor_copy(sel_f32[64:65, 64:128], ones_f32[0:1, 0:64])
        nc.vector.tensor_copy(SEL, sel_f32)
        nc.vector.memset(LR2[0], 0.0)
        nc.vector.memset(LR2[1], 0.0)

        # V-augmentation ones columns
        ones_col = ones_f32[:, 0:8].rearrange("p (h o) -> p h o", o=1)
        for tb in range(16):
            vdst = VA[tb].rearrange("p (h e) -> p h e", e=128)[:, :, 64:65]
            nc.vector.tensor_copy(vdst, ones_col)

        # ---------------- phase 0: V ----------------
        v_done = [0]

        def make_v_units():
            units = []

            def unit_tb(tb):
                def unit():
                    ps = pfl.tile([128, 512], F32, name="fill", tag="fill")
                    for c in range(8):
                        nc.tensor.matmul(
                            ps,
                            XTc(c, tb * 128, (tb + 1) * 128),
                            WVall[:, c * G : (c + 1) * G],
                            start=(c == 0),
                            stop=(c == 7),
                        )
                    vdst = VA[tb].rearrange("p (h e) -> p h e", e=128)[
                        :, :, 0:64
                    ]
                    nc.vector.tensor_copy(
                        vdst, ps.rearrange("p (h d) -> p h d", d=64)
                    )
                    v_done[0] += 1
                return unit

            for tb in range(16):
                units.append(unit_tb(tb))
            return units

        # ---------------- QK machinery ----------------
        def emit_w_slices(hp, engine_q, engine_k):
            wqh = wsl.tile([128, 8 * 128], BF16, name="wqh", tag="wqh")
            wkh = wsl.tile([128, 8 * 128], BF16, name="wkh", tag="wkh")
            engine_q.dma_start(
                out=wqh.rearrange("p (c h) -> p c h", h=128),
                in_=wq.rearrange("(c p) g -> p c g", p=128)[
                    :, :, hp * 128 : (hp + 1) * 128
                ],
            )
            engine_k.dma_start(
                out=wkh.rearrange("p (c h) -> p c h", h=128),
                in_=wk.rearrange("(c p) g -> p c g", p=128)[
                    :, :, hp * 128 : (hp + 1) * 128
                ],
            )
            return {"q": wqh, "k": wkh}

        def make_qk_units(hp, wtiles):
            """QK projection split into half-units (4 matmuls each) for
            fine-grained filler pacing."""
            qt = qtkt.tile([128, T], BF16, name="qtP", tag="qtP")
            kt = qtkt.tile([128, T], BF16, name="ktP", tag="ktP")
            units = []
            for t4 in range(4):
                for mat, dst in (("q", qt), ("k", kt)):
                    box = {}
                    wt = wtiles[mat]

                    def unit_a(wt=wt, t4=t4, box=box):
                        ps = pfl.tile([128, 512], F32, name="fill", tag="fill")
                        box["ps"] = ps
                        for c in range(4):
                            nc.tensor.matmul(
                                ps,
                                wt[:, c * 128 : (c + 1) * 128],
                                XTc(c, t4 * 512, (t4 + 1) * 512),
                                start=(c == 0),
                                stop=False,
                            )

                    def unit_b(wt=wt, dst=dst, t4=t4, box=box):
                        ps = box["ps"]
                        for c in range(4, 8):
                            nc.tensor.matmul(
                                ps,
                                wt[:, c * 128 : (c + 1) * 128],
                                XTc(c, t4 * 512, (t4 + 1) * 512),
                                start=False,
                                stop=(c == 7),
                            )
                        nc.vector.tensor_copy(
                            dst[:, t4 * 512 : (t4 + 1) * 512], ps
                        )

                    units.append(unit_a)
                    units.append(unit_b)
            return qt, kt, units

        # ---------- proj units (tail / fillers for pair 3) ----------
        def proj_units(tb):
            ot = {}
            def unit_ch(ch):
                def unit():
                    if ch == 0:
                        ot["t"] = osb.tile([128, C], BF16, name="ot", tag="ot")
                    ps = pfl.tile([128, 512], F32, name="fill", tag="fill")
                    for cb in range(4):
                        nc.tensor.matmul(
                            ps,
                            YT[cb][:, tb * 128 : (tb + 1) * 128],
                            WPall[:, cb * C + ch * 512 : cb * C + (ch + 1) * 512],
                            start=(cb == 0),
                            stop=(cb == 3),
                        )
                    nc.vector.tensor_copy(
                        ot["t"][:, ch * 512 : (ch + 1) * 512], ps
                    )
                    nc.sync.dma_start(
                        out=out[
                            tb * 128 : (tb + 1) * 128,
                            ch * 512 : (ch + 1) * 512,
                        ],
                        in_=ot["t"][:, ch * 512 : (ch + 1) * 512],
                    )
                return unit
            return [unit_ch(0), unit_ch(1)]

        def tail_units(qc):
            units = []
            for tb in range(qc * 4, qc * 4 + 4):
                units.extend(proj_units(tb))
            return units

        # ---------------- attention ----------------
        fill_q = deque()
        reserve = []
        credit = [0.0]
        remkb = [40]

        def pump(n):
            for _ in range(min(n, len(fill_q))):
                fill_q.popleft()()

        def pump_paced():
            # adaptive: spread the current queue over the k-blocks left in
            # this head-pair so the PE never starves near the hp boundary
            if remkb[0] > 0:
                credit[0] += len(fill_q) / remkb[0]
                remkb[0] -= 1
            n = int(credit[0])
            if n > 0:
                n = min(n, len(fill_q))
                credit[0] -= n
                pump(n)

        def flush_carry(carry, n=None):
            """Emit up to n (or all) deferred AVs from the previous qc; when
            the queue empties, emit that qc's drain + norm."""
            if carry is None:
                return None
            cpend, cemit, cfin = carry
            k = len(cpend) if n is None else min(n, len(cpend))
            for _ in range(k):
                cemit(*cpend.popleft())
                if n is None and len(cpend) % 2 == 1:
                    pump(1)
            if not cpend:
                cfin()
                return None
            return carry

        def attention(hp, qt, kt, qc, carry):
            q0 = qc * QCH
            nkb = (qc + 1) * 4
            hA, hB = 2 * hp, 2 * hp + 1
            # psy tiles allocated LAZILY at the first AV emission: eager
            # allocation here would predate the carried-in previous qc's AV
            # writes + drain reads of the same (bufs=1) buffers, and the
            # pool's WAR tracking would miss them -> race
            ytbox = {}

            def get_yts():
                if "A" not in ytbox:
                    ytbox["A"] = psy.tile(
                        [128, QCH], F32, name="ytA", tag="ytA"
                    )
                    ytbox["B"] = psy.tile(
                        [128, QCH], F32, name="ytB", tag="ytB"
                    )
                return ytbox["A"], ytbox["B"]

            def emit_av(idx, pt, off, w, kb):
                ytA, ytB = get_yts()
                nc.tensor.matmul(
                    ytA[:, off : off + w],
                    VA[kb][:, hA * 128 : hA * 128 + 128],
                    pt[:, off : off + w],
                    start=(idx == 0),
                    stop=(idx == nkb - 1),
                )
                nc.tensor.matmul(
                    ytB[:, off : off + w],
                    VA[kb][:, hB * 128 : hB * 128 + 128],
                    pt[:, 512 + off : 512 + off + w],
                    start=(idx == 0),
                    stop=(idx == nkb - 1),
                )

            pend = deque()
            # absorb the qc-start scalar backlog (previous qc's flush exps)
            pump(2)
            # diagonal (masked) blocks early -- their gpsimd mask-muls must
            # finish before their deferred AVs -- but SPACED among full
            # blocks: emitting all four trimmed diag S's back-to-back floods
            # the scalar queue with a burst of exps and the early AVs stall
            diag = list(range(qc * 4, nkb))
            full = list(range(0, qc * 4))
            kb_order = []
            for i in range(max(len(diag), len(full))):
                if i < len(diag):
                    kb_order.append(diag[i])
                if i < len(full):
                    kb_order.append(full[i])
            for idx, kb in enumerate(kb_order):
                j = kb - qc * 4
                off = j * 128 if j >= 1 else 0
                w = 512 - off
                ksl = slice(kb * KBLK, (kb + 1) * KBLK)
                sAB = pss.tile([128, 1024], F32, name="sAB", tag="sAB")
                nc.tensor.matmul(
                    sAB[:, off : 512],
                    kt[0:64, ksl],
                    qt[0:64, q0 + off : q0 + QCH],
                    start=True,
                    stop=True,
                    tile_position=(0, 0),
                )
                nc.tensor.matmul(
                    sAB[:, 512 + off : 1024],
                    kt[64:128, ksl],
                    qt[64:128, q0 + off : q0 + QCH],
                    start=True,
                    stop=True,
                    tile_position=(64, 0),
                )
                pt = ptp.tile([128, 1024], BF16, name="pt", tag="pt")
                nc.scalar.activation(
                    pt[:, off:1024], sAB[:, off:1024], EXP, scale=0.125
                )
                if j >= 0:
                    pv = pt.rearrange("p (s q) -> p s q", s=2)[
                        :, :, off : off + 128
                    ]
                    nc.gpsimd.tensor_mul(
                        pv, pv, MSK.rearrange("p (s q) -> p s q", s=2)
                    )
                pump_paced()
                carry = flush_carry(carry, 2)
                if len(pend) == AVLAG:
                    emit_av(*pend.popleft())
                pend.append((idx, pt, off, w, kb))

            def finish(hp=hp, qc=qc, q0=q0):
                ytA, ytB = get_yts()
                lr2 = LR2[(hp * 4 + qc) % 2]
                yslA = YT[hp][0:64, q0 : q0 + QCH]
                yslB = YT[hp][64:128, q0 : q0 + QCH]
                for sub, (yt, ysl) in enumerate(((ytA, yslA), (ytB, yslB))):
                    nc.vector.tensor_copy(ysl, yt[0:64, :])
                    lf = nrm.tile([1, 512], F32, name="lf", tag="lf")
                    nc.vector.tensor_copy(lf, yt[64:65, :])
                    lf2 = nrm.tile([1, 512], F32, name="lf2", tag="lf2")
                    nc.vector.reciprocal_approx_fast(lf2, lf)
                    nc.vector.tensor_copy(
                        lr2[sub * 64 : sub * 64 + 1, :], lf2
                    )

                def norm_fin(yslA=yslA, yslB=yslB, lr2=lr2, hp=hp, qc=qc):
                    rb = pfl.tile([128, 512], F32, name="fill", tag="fill")
                    nc.tensor.matmul(rb, SEL, lr2, start=True, stop=True)
                    nc.vector.tensor_mul(yslA, yslA, rb[0:64, :])
                    nc.vector.tensor_mul(yslB, yslB, rb[64:128, :])
                    if hp == 3:
                        # emitting the proj tail here (not earlier) makes
                        # "proj reads YT only after it is drained+normalized"
                        # hold by construction; 4 of qc3's units are held in
                        # reserve to cover the final qc0 drain-chain latency
                        units = tail_units(qc)
                        if qc == 3:
                            reserve.extend(units[4:])
                            units = units[:4]
                        fill_q.extend(units)

                fill_q.appendleft(norm_fin)

            return (pend, emit_av, finish)

        # ---------------- main schedule ----------------
        wtiles0 = emit_w_slices(0, nc.sync, nc.sync)
        nc.sync.dma_start(
            out=WPall.rearrange("p (b c) -> p b c", c=C),
            in_=wp.rearrange("(b p) c -> p b c", p=128),
        )
        v_units = make_v_units()
        for u in v_units:
            u()
        qt, kt, units = make_qk_units(0, wtiles0)
        for u in units:
            u()
        carry = None
        for hp in range(4):
            nqt = nkt = None
            remkb[0] = 40
            credit[0] = 0.0
            if hp < 3:
                nwt = emit_w_slices(hp + 1, nc.sync, nc.sync)
                nqt, nkt, nunits = make_qk_units(hp + 1, nwt)
                fill_q.extend(nunits)
            for qc in (3, 2, 1, 0):
                carry = attention(hp, qt, kt, qc, carry)
            if hp < 3:
                pump(len(fill_q))
                qt, kt = nqt, nkt
        carry = flush_carry(carry)
        # ready proj work covers the final drain/norm DVE chain latency
        for u in reserve:
            u()
        while fill_q:
            fill_q.popleft()()

    nc.compile()
    return nc


_NC_CACHE = None


def kernel(x0, w_attn, w_proj, _trace=False, _tmpdir=None):
    global _NC_CACHE
    import ml_dtypes

    from concourse.bass_utils import run_bass_kernel_spmd

    BF = ml_dtypes.bfloat16
    x0 = np.asarray(x0, dtype=np.float32)
    w_attn = np.asarray(w_attn, dtype=np.float32)
    w_proj = np.asarray(w_proj, dtype=np.float32)
    B = x0.shape[0]

    if _NC_CACHE is None:
        _NC_CACHE = _build_nc()
    nc = _NC_CACHE

    tri = np.triu(np.ones((128, 128), dtype=np.float32))
    msk = np.concatenate([tri, tri], axis=1).astype(BF)
    in_maps = []
    for core in range(8):
        b, g = divmod(core, 2)
        in_maps.append(
            {
                "xT": np.ascontiguousarray(x0[b].T).astype(BF),
                "wq": np.ascontiguousarray(
                    w_attn[:, g * G : (g + 1) * G]
                ).astype(BF),
                "wk": np.ascontiguousarray(
                    w_attn[:, C + g * G : C + (g + 1) * G]
                ).astype(BF),
                "wv": np.ascontiguousarray(
                    w_attn[:, 2 * C + g * G : 2 * C + (g + 1) * G]
                ).astype(BF),
                "wp": np.ascontiguousarray(
                    w_proj[g * G : (g + 1) * G, :]
                ).astype(BF),
                "mask": msk,
            }
        )

    res = run_bass_kernel_spmd(
        nc, in_maps, list(range(8)), trace=_trace, tmpdir=_tmpdir
    )
    outp = np.empty((B, T, C), dtype=np.float32)
    for b in range(B):
        outp[b] = res.results[2 * b]["out"].astype(np.float32) + res.results[
            2 * b + 1
        ]["out"].astype(np.float32)
    if _trace:
        kernel.last_exec_time_ns = res.exec_time_ns
    return outp
